# revision 1
# baseline (speedup 1.0000x reference)
"""Multi-head self-attention (B=2, T=2048, D=2048, H=16, RoPE, causal)
as a Bass/Tile kernel running SPMD on 8 trn2 NeuronCores.

Sharding: tensor-parallel over heads (2 heads per core). Each core
computes its heads' Q/K/V projections, RoPE, causal attention, and a
partial out-projection over its 256 feature columns; the host sums the
8 partial outputs (all-reduce equivalent).

Dataflow (per core, per batch):
  - projections in "T-layout" (feature dim on partitions, time on free):
    qT/kT = RoPE(W x^T); V transposed to natural layout via PE.
  - scores computed directly transposed: S^T[tk, tq] = K_j^T.T-free ...
    one 128-contraction matmul per (key-chunk, 512-wide q-group), so the
    exp output P^T is immediately the PV matmul's moving operand --
    no P transposes. Softmax skips max-subtraction (scores are O(10)).
  - row sums Z via a [128,1] ones matmul accumulated in PSUM across
    key chunks; reciprocal on DVE; partition-broadcast on GpSimd;
    normalization fused with the PSUM->SBUF move of the PV result.
  - out-projection accumulates the two head-chunks in PSUM, partial
    result DMA'd out; host sums partials across cores.
"""

import sys

sys.path.insert(0, "/opt/trn_rl_repo")

import ml_dtypes
import numpy as np

import concourse.bass as bass
import concourse.bass_isa as bass_isa
import concourse.mybir as mybir
import concourse.tile as tile
from concourse.bass_utils import run_bass_kernel_spmd
from concourse.tile_rust import add_dep_helper


def _absorb(eng, producers):
    """Emit engine-nops sync-depending on <=2 producers each so the engine
    observes those semaphores; later same-engine instructions then elide
    the waits (ISA instructions carry at most ~2 wait slots). Returns the
    nops; order consumers after them with _after()."""
    producers = [p for p in producers if p is not None]
    nops = []
    for i in range(0, len(producers), 2):
        nop = eng.nop()
        for p in producers[i : i + 2]:
            add_dep_helper(nop.ins, p.ins, sync=True, reason="wait-absorb")
        nops.append(nop)
    return nops


def _after(inst, nops):
    """Order `inst` after absorber nops (scheduler-only edges)."""
    for nop in nops:
        add_dep_helper(inst.ins, nop.ins, sync=False, reason="wait-absorb-order")


_SPILL_TYPES = (
    "InstDMACopy",
    "InstMatmult",
    "InstLdweights",
    "InstNoOp",
    "InstMemset",
    "InstPartitionBroadcast",
    "InstPartitionAllReduce",
    "InstTensorScalarAffineSelect",
)


def _legalize_waits(nc):
    """Walrus codegen rejects >2 sync waits on DMA/matmul/nop-class
    instructions, and Tile's pool-recycle waits bypass its own elision.
    Two fixes, both sound w.r.t. per-engine program order:
      1. strip waits already dominated by an earlier same-engine wait
      2. spill excess waits (>2) onto freshly inserted same-engine NoOps
         placed immediately before the offending instruction
    """
    # NOTE: wait-stripping by same-engine dominance is UNSOUND here --
    # Tile recycles semaphores mid-kernel (pool close -> sem_clear), so
    # values are not monotonic. Only the spill transformation is safe.
    do_strip = False
    do_spill = True
    seen = {}  # engine -> {sem_name: max_waited_value}
    spill_id = [0]
    for bb in nc.m.functions[0].blocks:
        new_insts = []
        for inst in bb.instructions:
            si = getattr(inst, "sync_info", None)
            if si is None or not si.on_wait:
                new_insts.append(inst)
                continue
            eng = getattr(inst, "engine", None)
            emap = seen.setdefault(str(eng), {})
            kept = []
            for w in si.on_wait:
                if (
                    w.sync_type == "semaphore"
                    and w.wait_mode == "sem-ge-imm"
                    and w.wait_reg is None
                    and w.wait_value is not None
                ):
                    if do_strip and emap.get(w.ant_name, -1) >= w.wait_value:
                        continue  # dominated: drop
                    emap[w.ant_name] = w.wait_value
                kept.append(w)
            if do_spill and len(kept) > 1 and eng is not None:
                excess, kept = kept[:-1], kept[-1:]
                for w in excess:
                    spill_id[0] += 1
                    nop = mybir.InstNoOp(
                        name=f"I-wspill-{spill_id[0]}",
                        ins=[],
                        outs=[],
                        engine=eng,
                    )
                    nop.sync_info = mybir.SyncInfo(on_wait=[w], on_update=[])
                    new_insts.append(nop)
            if len(kept) != len(si.on_wait):
                si.on_wait[:] = kept
            new_insts.append(inst)
        if len(new_insts) != len(bb.instructions):
            bb.instructions[:] = new_insts

B, T, D, H, HD = 2, 2048, 2048, 16, 128
NCORES = 8
HPC = H // NCORES            # heads per core = 2
M_PC = HPC * HD              # per-core feature slice = 256
BT = B * T                   # 4096
SCALE = HD ** -0.5
ROPE_THETA = 10000.0

F32 = mybir.dt.float32
BF16 = mybir.dt.bfloat16
BF16_NP = ml_dtypes.bfloat16

TB = 512                     # t-block for projections / q-groups
NTB_B = T // TB              # 4 t-blocks per batch
NMC = D // 128               # 16 contraction chunks
NKC = T // 128               # 16 key chunks per batch


def build_program():
    nc = bass.Bass()

    xT_d = nc.declare_dram_parameter("xT", [D, BT], BF16, isOutput=False)
    perm_d = nc.declare_dram_parameter("permM", [HD, HD], BF16, isOutput=False)
    negm_d = nc.declare_dram_parameter("negmM", [128, 128], F32, isOutput=False)
    fneg_d = nc.declare_dram_parameter("fnegM", [128, 128], F32, isOutput=False)
    id_d = nc.declare_dram_parameter("identM", [128, 128], BF16, isOutput=False)
    wq_d = nc.declare_dram_parameter("wqT", [D, M_PC], BF16, isOutput=False)
    wk_d = nc.declare_dram_parameter("wkT", [D, M_PC], BF16, isOutput=False)
    wv_d = nc.declare_dram_parameter("wvT", [D, M_PC], BF16, isOutput=False)
    wo_d = nc.declare_dram_parameter("woT", [M_PC, D], BF16, isOutput=False)
    cos_d = nc.declare_dram_parameter("cosT", [HD, T], F32, isOutput=False)
    sinh_d = nc.declare_dram_parameter("sinhT", [HD, T], F32, isOutput=False)
    out_d = nc.declare_dram_parameter("partialT", [D, BT], F32, isOutput=True)

    xT_v = xT_d.rearrange("(c p) t -> p c t", p=128)      # [128, 16, BT]
    wq_v = wq_d.rearrange("(c p) n -> p c n", p=128)      # [128, 16, 256]
    wk_v = wk_d.rearrange("(c p) n -> p c n", p=128)
    wv_v = wv_d.rearrange("(c p) n -> p c n", p=128)
    wo_v = wo_d.rearrange("(c p) n -> p c n", p=128)      # [128, 2, 2048]
    out_v = out_d.rearrange("(c p) t -> p c t", p=128)    # [128, 16, BT]

    with tile.TileContext(nc) as tc:
        with (
            tc.tile_pool(name="wpool", bufs=1) as wpool,
            tc.tile_pool(name="big", bufs=1) as big,
            tc.tile_pool(name="rp", bufs=2) as rp,
            tc.tile_pool(name="attn_sb", bufs=3) as asb,
            tc.tile_pool(name="z_sb", bufs=2) as zsb,
            tc.tile_pool(name="fs_sb", bufs=4) as fsb,
        ):
            # ---- constants / weights ----
            wq_sb = wpool.tile([128, NMC, M_PC], BF16, tag="wq")
            wk_sb = wpool.tile([128, NMC, M_PC], BF16, tag="wk")
            wv_sb = wpool.tile([128, NMC, M_PC], BF16, tag="wv")
            nc.sync.dma_start(out=wq_sb, in_=wq_v)
            nc.sync.dma_start(out=wk_sb, in_=wk_v)
            nc.sync.dma_start(out=wv_sb, in_=wv_v)
            cos_sb = wpool.tile([128, T], F32, tag="cos")
            sinh_sb = wpool.tile([128, T], F32, tag="sinh")
            nc.sync.dma_start(out=cos_sb, in_=cos_d[:, :])
            nc.sync.dma_start(out=sinh_sb, in_=sinh_d[:, :])
            perm_sb = wpool.tile([HD, HD], BF16, tag="perm")
            nc.sync.dma_start(out=perm_sb, in_=perm_d[:, :])

            # constant tiles (host-provided; gpsimd is avoided entirely --
            # its tail sem-clear ISA doesn't encode on this toolchain)
            negm = wpool.tile([128, 128], F32, tag="negm")
            nc.sync.dma_start(out=negm, in_=negm_d[:, :])
            fullneg = wpool.tile([128, 128], F32, tag="fullneg")
            nc.sync.dma_start(out=fullneg, in_=fneg_d[:, :])
            ident = wpool.tile([128, 128], BF16, tag="ident")
            nc.sync.dma_start(out=ident, in_=id_d[:, :])
            ones_col = wpool.tile([128, 1], BF16, tag="ones_c")
            nc.vector.memset(ones_col, 1.0)
            ones_row = wpool.tile([1, 128], F32, tag="ones_r")
            nc.vector.memset(ones_row, 1.0)

            wo_sb = wpool.tile([128, HPC, D], BF16, tag="wo")
            wo_dma = nc.sync.dma_start(out=wo_sb, in_=wo_v)

            prev_x_dmas = []
            for b in range(B):
                t0 = b * T  # global t offset of this batch

                # persistent per-batch tensors (slots reused across b)
                qT = big.tile([128, HPC, T], BF16, tag="qT")   # [hd, h, t]
                kT = big.tile([128, HPC, T], BF16, tag="kT")
                vN = big.tile([128, HPC, NKC, HD], BF16, tag="vN")  # [tk, h, j, d]
                oT = big.tile([128, HPC, T], BF16, tag="oT")   # attn out, T-layout

                # ---------------- projections + RoPE ----------------
                # whole-batch x resident in SBUF, loaded as 16 disjoint
                # sub-DMAs (keeps per-DMA sync waits low). Before the slot
                # is recycled for batch 1, let SP observe batch 0's DMA
                # queue semaphores so the WAW waits collapse.
                xnops = _absorb(nc.sync, prev_x_dmas)
                xb = big.tile([128, NMC, T], BF16, tag="xb")
                prev_x_dmas = []
                for mc in range(NMC):
                    d = nc.sync.dma_start(
                        out=xb[:, mc, :], in_=xT_v[:, mc, t0 : t0 + T]
                    )
                    _after(d, xnops)
                    prev_x_dmas.append(d)
                with tc.tile_pool(name="proj_ps", bufs=1, space="PSUM") as pps, \
                     tc.tile_pool(name="vt_ps", bufs=1, space="PSUM") as vtp:
                    for tb in range(NTB_B):
                        ts_l = slice(tb * TB, (tb + 1) * TB)          # in-batch
                        ps = {}
                        for h in range(HPC):
                            ps["q", h] = pps.tile(
                                [128, TB], F32, tag=f"q{h}", name=f"ps_q{h}"
                            )
                            ps["k", h] = pps.tile(
                                [128, TB], F32, tag=f"k{h}", name=f"ps_k{h}"
                            )
                            ps["v", h] = pps.tile(
                                [128, TB], F32, tag=f"v{h}", name=f"ps_v{h}"
                            )
                        for mc in range(NMC):
                            for h in range(HPC):
                                hs = slice(h * HD, (h + 1) * HD)
                                for nm, wsb in (
                                    ("q", wq_sb),
                                    ("k", wk_sb),
                                    ("v", wv_sb),
                                ):
                                    nc.tensor.matmul(
                                        ps[nm, h],
                                        lhsT=wsb[:, mc, hs],
                                        rhs=xb[:, mc, ts_l],
                                        start=(mc == 0),
                                        stop=(mc == NMC - 1),
                                    )
                        for h in range(HPC):
                            # RoPE for q, k: half-swap via PE permutation
                            # matmul; fp32 combine on DVE; bf16 result
                            for nm, dest in (("q", qT), ("k", kT)):
                                raw = rp.tile([128, TB], BF16, tag="raw")
                                nc.scalar.activation(
                                    raw, ps[nm, h], mybir.ActivationFunctionType.Copy
                                )
                                swps = vtp.tile([128, TB], F32, tag="swps")
                                nc.tensor.matmul(
                                    swps, lhsT=perm_sb, rhs=raw,
                                    start=True, stop=True,
                                )
                                t1 = rp.tile([128, TB], F32, tag="t1")
                                nc.vector.tensor_mul(t1, raw, cos_sb[:, ts_l])
                                t2 = rp.tile([128, TB], F32, tag="t2")
                                nc.vector.tensor_mul(t2, swps, sinh_sb[:, ts_l])
                                nc.vector.tensor_add(dest[:, h, ts_l], t1, t2)
                            # V: cast to bf16 SBUF then PE-transpose to natural
                            vt_sb = rp.tile([128, TB], BF16, tag="vtmp")
                            nc.scalar.activation(
                                vt_sb, ps["v", h], mybir.ActivationFunctionType.Copy
                            )
                            for s in range(TB // 128):
                                j = tb * (TB // 128) + s
                                pst = vtp.tile([128, 128], BF16, tag="vt")
                                nc.tensor.transpose(
                                    pst, vt_sb[:, s * 128 : (s + 1) * 128], ident
                                )
                                nc.vector.tensor_copy(vN[:, h, j, :], pst)

                # ---------------- attention ----------------
                last_exp = last_omul = None
                with tc.tile_pool(name="st_ps", bufs=3, space="PSUM") as stp, \
                     tc.tile_pool(name="pv_ps", bufs=2, space="PSUM") as pvp, \
                     tc.tile_pool(name="z_ps", bufs=1, space="PSUM") as zpp:
                    for h in range(HPC):
                        for qg in range(NTB_B):
                            qs = slice(qg * TB, (qg + 1) * TB)  # in-batch q range
                            jmax = (TB // 128) * (qg + 1)
                            po = pvp.tile([128, TB], F32, tag="po")
                            zrow = zpp.tile([1, TB], F32, tag="zrow")
                            for j in range(jmax):
                                ks_ = slice(j * 128, (j + 1) * 128)
                                st = stp.tile([128, TB], F32, tag="st")
                                nc.tensor.matmul(
                                    st,
                                    lhsT=kT[:, h, ks_],
                                    rhs=qT[:, h, qs],
                                    start=True,
                                    stop=True,
                                )
                                # causal mask: diag triangle / full block
                                for i in range(TB // 128):
                                    qb = qg * (TB // 128) + i
                                    if j == qb or j > qb:
                                        blk = slice(i * 128, (i + 1) * 128)
                                        nc.vector.tensor_add(
                                            st[:, blk],
                                            st[:, blk],
                                            negm if j == qb else fullneg,
                                        )
                                pt = asb.tile([128, TB], BF16, tag="pt")
                                last_exp = nc.scalar.activation(
                                    pt,
                                    st,
                                    mybir.ActivationFunctionType.Exp,
                                    scale=SCALE,
                                )
                                nc.tensor.matmul(
                                    zrow,
                                    lhsT=ones_col,
                                    rhs=pt,
                                    start=(j == 0),
                                    stop=(j == jmax - 1),
                                )
                                nc.tensor.matmul(
                                    po,
                                    lhsT=vN[:, h, j, :],
                                    rhs=pt,
                                    start=(j == 0),
                                    stop=(j == jmax - 1),
                                )
                            # normalization: recip + PE ones-matmul broadcast
                            zrs = zsb.tile([1, TB], F32, tag="zrs")
                            nc.vector.reciprocal(zrs, zrow)
                            zbp = zpp.tile([128, TB], F32, tag="zbp")
                            nc.tensor.matmul(
                                zbp, lhsT=ones_row, rhs=zrs,
                                start=True, stop=True,
                            )
                            zbr = zsb.tile([128, TB], F32, tag="zbr")
                            nc.scalar.activation(
                                zbr, zbp, mybir.ActivationFunctionType.Copy
                            )
                            last_omul = nc.vector.tensor_mul(
                                oT[:, h, qs], po, zbr
                            )

                # ---------------- out-projection (partial) ----------------
                # let PE observe the attention-phase tail so the first
                # out-proj matmuls don't aggregate >2 waits
                onops = _absorb(nc.tensor, [last_exp, last_omul, wo_dma])
                with tc.tile_pool(name="fo_ps", bufs=1, space="PSUM") as fop:
                    for nb in range(D // 128):
                        nbs = slice(nb * 128, (nb + 1) * 128)
                        fo = {}
                        for m in range(HPC):
                            for tb in range(NTB_B):
                                tbs = slice(tb * TB, (tb + 1) * TB)
                                if m == 0:
                                    fo[tb] = fop.tile(
                                        [128, TB], F32, tag=f"fo{tb % 4}",
                                        name=f"fo{tb}",
                                    )
                                mm = nc.tensor.matmul(
                                    fo[tb],
                                    lhsT=wo_sb[:, m, nbs],
                                    rhs=oT[:, m, tbs],
                                    start=(m == 0),
                                    stop=(m == HPC - 1),
                                )
                                if nb == 0 and m == 0:
                                    _after(mm, onops)
                        for tb in range(NTB_B):
                            fs = fsb.tile([128, TB], F32, tag=f"fs{tb}")
                            if tb % 2 == 0:
                                nc.vector.tensor_copy(fs, fo[tb])
                            else:
                                nc.scalar.activation(
                                    fs, fo[tb], mybir.ActivationFunctionType.Copy
                                )
                            nc.sync.dma_start(
                                out=out_v[:, nb, t0 + tb * TB : t0 + (tb + 1) * TB],
                                in_=fs,
                            )
    _legalize_waits(nc)
    return nc


_NC_CACHE = None


def _get_program():
    global _NC_CACHE
    if _NC_CACHE is None:
        _NC_CACHE = build_program()
    return _NC_CACHE


def _rope_tables():
    inv_freq = 1.0 / (ROPE_THETA ** (np.arange(0, HD, 2, dtype=np.float32) / HD))
    freqs = np.arange(T, dtype=np.float32)[:, None] * inv_freq[None, :]  # (T, 64)
    emb = np.concatenate([freqs, freqs], axis=-1)                        # (T, 128)
    cosT = np.ascontiguousarray(np.cos(emb).T.astype(np.float32))        # [128, T]
    sinT = np.sin(emb).T.astype(np.float32)
    sinhT = np.ascontiguousarray(
        np.concatenate([-sinT[: HD // 2], sinT[HD // 2 :]], axis=0)
    )
    return cosT, sinhT


def kernel(x, Wq, Wk, Wv, Wo, **run_kwargs):
    x = np.asarray(x, dtype=np.float32)
    Wq = np.asarray(Wq, dtype=np.float32)
    Wk = np.asarray(Wk, dtype=np.float32)
    Wv = np.asarray(Wv, dtype=np.float32)
    Wo = np.asarray(Wo, dtype=np.float32)

    nc = _get_program()
    cosT, sinhT = _rope_tables()
    xT = np.ascontiguousarray(x.reshape(BT, D).T).astype(BF16_NP)  # [D, BT]
    permM = np.zeros((HD, HD), dtype=BF16_NP)
    for m in range(HD):
        permM[(m + HD // 2) % HD, m] = 1.0  # out[m] = in[(m+64)%128]
    # S^T[tk, tq] causal masks: keep where tq(col) >= tk(row)
    r = np.arange(128)
    negmM = np.where(r[None, :] >= r[:, None], 0.0, -1e30).astype(np.float32)
    fnegM = np.full((128, 128), -1e30, dtype=np.float32)
    identM = np.eye(128, dtype=BF16_NP)

    in_maps = []
    for c in range(NCORES):
        sl = slice(c * M_PC, (c + 1) * M_PC)
        in_maps.append(
            {
                "xT": xT,
                "permM": permM,
                "negmM": negmM,
                "fnegM": fnegM,
                "identM": identM,
                "wqT": np.ascontiguousarray(Wq[sl, :].T).astype(BF16_NP),
                "wkT": np.ascontiguousarray(Wk[sl, :].T).astype(BF16_NP),
                "wvT": np.ascontiguousarray(Wv[sl, :].T).astype(BF16_NP),
                "woT": np.ascontiguousarray(Wo[:, sl].T).astype(BF16_NP),
                "cosT": cosT,
                "sinhT": sinhT,
            }
        )

    res = run_bass_kernel_spmd(nc, in_maps, list(range(NCORES)), **run_kwargs)
    acc = np.zeros((D, BT), dtype=np.float32)
    for c in range(NCORES):
        acc += res.results[c]["partialT"]
    out = np.ascontiguousarray(acc.T).reshape(B, T, D)
    if run_kwargs:
        return out, res
    return out



# revision 15
# speedup vs baseline: 1.3899x; 1.3899x over previous
"""Multi-head self-attention (B=2, T=2048, D=2048, H=16, RoPE, causal)
as a Bass/Tile kernel running SPMD on 8 trn2 NeuronCores.

Sharding: tensor-parallel over heads (2 heads per core). Each core
computes its heads' Q/K/V projections, RoPE, causal attention, and a
partial out-projection over its 256 feature columns; the host sums the
8 partial outputs (all-reduce equivalent).

Dataflow (per core, per batch):
  - q/k projections in "T-layout" (feature dim on partitions, time on
    free); V projected directly into natural [t, d] layout by using the
    x tile as the stationary operand (no PE transposes).
  - RoPE with the head_dim rows interleaved as (r, r+64) pairs, a
    permutation folded into Wq/Wk and the cos/sin tables host-side (the
    QK^T contraction is invariant); the half-rotation is then a swap of
    adjacent partitions done with a DVE stream-shuffle (no PE matmul).
  - scores computed directly transposed: S^T[tk, tq] = K_j^T.T @ Q,
    one 128-contraction matmul per (key-chunk, 512-wide q-group), so the
    exp output P^T is immediately the PV matmul's moving operand.
    Column-blocks that the causal mask fully zeroes are skipped (matmul/
    exp widths narrowed to the valid range). Softmax skips
    max-subtraction (scores are O(10)).
  - row sums Z via a [128,1] ones matmul accumulated in PSUM across key
    chunks; reciprocal on DVE; partition-broadcast via a bf16 ones
    matmul; normalization fused with the PSUM->SBUF move of the PV
    result.
  - out-projection (bf16 partial result) is emitted as filler work
    inside the next q-group's attention loops to keep the PE busy while
    exp results are in flight; host sums partials across cores.
"""

import sys

sys.path.insert(0, "/opt/trn_rl_repo")

import ml_dtypes
import numpy as np

import concourse.bass as bass
import concourse.mybir as mybir
import concourse.tile as tile
from concourse.bass_utils import run_bass_kernel_spmd
from concourse.tile_rust import add_dep_helper


def _legalize_waits(nc):
    """Walrus codegen rejects >2 sync waits on DMA/matmul/nop-class
    instructions, and Tile's pool-recycle waits bypass its own elision.
    Spill excess waits (>1) onto freshly inserted same-engine NoOps
    placed immediately before the offending instruction (sound w.r.t.
    per-engine program order)."""
    spill_id = [0]
    for bb in nc.m.functions[0].blocks:
        new_insts = []
        for inst in bb.instructions:
            si = getattr(inst, "sync_info", None)
            if si is None or not si.on_wait:
                new_insts.append(inst)
                continue
            eng = getattr(inst, "engine", None)
            kept = list(si.on_wait)
            if len(kept) > 1 and eng is not None:
                excess, kept = kept[:-1], kept[-1:]
                for w in excess:
                    spill_id[0] += 1
                    nop = mybir.InstNoOp(
                        name=f"I-wspill-{spill_id[0]}",
                        ins=[],
                        outs=[],
                        engine=eng,
                    )
                    nop.sync_info = mybir.SyncInfo(on_wait=[w], on_update=[])
                    new_insts.append(nop)
                si.on_wait[:] = kept
            new_insts.append(inst)
        if len(new_insts) != len(bb.instructions):
            bb.instructions[:] = new_insts


B, T, D, H, HD = 2, 2048, 2048, 16, 128
NCORES = 8
HPC = H // NCORES            # heads per core = 2
M_PC = HPC * HD              # per-core feature slice = 256
BT = B * T                   # 4096
SCALE = HD ** -0.5
ROPE_THETA = 10000.0

F32 = mybir.dt.float32
BF16 = mybir.dt.bfloat16
BF16_NP = ml_dtypes.bfloat16

TB = 512                     # t-block for projections / q-groups
NTB_B = T // TB              # 4 t-blocks per batch
NMC = D // 128               # 16 contraction chunks
NKC = T // 128               # 16 key chunks per batch
NNB = D // 128               # 16 out-proj row blocks
LOOK = 2                     # attention software-pipeline lookahead

# swap adjacent partitions within each 32-partition quadrant
SWAP_MASK = [i + 1 if i % 2 == 0 else i - 1 for i in range(32)]


def build_program():
    nc = bass.Bass()

    xT_d = nc.declare_dram_parameter("xT", [D, BT], BF16, isOutput=False)
    negmT_d = nc.declare_dram_parameter("negmTM", [128, 128], BF16, isOutput=False)
    id_d = nc.declare_dram_parameter("identM", [128, 128], BF16, isOutput=False)
    wq_d = nc.declare_dram_parameter("wqT", [D, M_PC], BF16, isOutput=False)
    wk_d = nc.declare_dram_parameter("wkT", [D, M_PC], BF16, isOutput=False)
    wv_d = nc.declare_dram_parameter("wvT", [D, M_PC], BF16, isOutput=False)
    wo_d = nc.declare_dram_parameter("woT", [M_PC, D], BF16, isOutput=False)
    cos_d = nc.declare_dram_parameter("cosT", [HD, T], BF16, isOutput=False)
    sinh_d = nc.declare_dram_parameter("sinhT", [HD, T], BF16, isOutput=False)
    out_d = nc.declare_dram_parameter("partialT", [D, BT], BF16, isOutput=True)

    xT_v = xT_d.rearrange("(c p) t -> p c t", p=128)      # [128, 16, BT]
    wq_v = wq_d.rearrange("(c p) n -> p c n", p=128)      # [128, 16, 256]
    wk_v = wk_d.rearrange("(c p) n -> p c n", p=128)
    wv_v = wv_d.rearrange("(c p) n -> p c n", p=128)
    wo_v = wo_d.rearrange("(c p) n -> p c n", p=128)      # [128, 2, 2048]
    out_v = out_d.rearrange("(c p) t -> p c t", p=128)    # [128, 16, BT]

    with tile.TileContext(nc) as tc:
        with (
            tc.tile_pool(name="wpool", bufs=1) as wpool,
            tc.tile_pool(name="big", bufs=1) as big,
            tc.tile_pool(name="obig", bufs=2) as obig,
            tc.tile_pool(name="xpool", bufs=2) as xpool,
            tc.tile_pool(name="rp", bufs=2) as rp,
            tc.tile_pool(name="attn_sb", bufs=4) as asb,
            tc.tile_pool(name="z_sb", bufs=2) as zsb,
            tc.tile_pool(name="fs_sb", bufs=2) as fsb,
            tc.tile_pool(name="fo_ps", bufs=2, space="PSUM") as fop,
        ):
            # ---- persistent weights / tables ----
            wq_sb = wpool.tile([128, NMC, M_PC], BF16, tag="wq")
            wk_sb = wpool.tile([128, NMC, M_PC], BF16, tag="wk")
            wv_sb = wpool.tile([128, NMC, M_PC], BF16, tag="wv")
            wo_sb = wpool.tile([128, HPC, D], BF16, tag="wo")
            cos_sb = wpool.tile([128, T], BF16, tag="cos")
            sinh_sb = wpool.tile([128, T], BF16, tag="sinh")
            negmT = wpool.tile([128, 128], BF16, tag="negmT")
            ident = wpool.tile([128, 128], BF16, tag="ident")
            ones_col = wpool.tile([128, 1], BF16, tag="ones_c")
            nc.vector.memset(ones_col, 1.0)
            ones_row = wpool.tile([1, 128], BF16, tag="ones_r")
            nc.vector.memset(ones_row, 1.0)
            # pre-warm the Act engine's Exp table during the head DMAs
            warm = wpool.tile([1, 2], F32, tag="warm")
            nc.vector.memset(warm, 0.0)
            warm2 = wpool.tile([1, 2], BF16, tag="warm2")
            nc.scalar.activation(warm2, warm, mybir.ActivationFunctionType.Exp)

            # ---- out-projection filler machinery ----
            filler_q = []
            fop_ref = [fop]
            fs_on_dve = [False]
            uid = [0]

            def emit_filler(n):
                for _ in range(min(n, len(filler_q))):
                    filler_q.pop(0)()

            def outproj_units(b, tb, oT0, oT1):
                """16 PE-units computing the partial out-projection for
                time block (b, tb); each unit = 2 accumulating matmuls +
                a PSUM->SBUF bf16 copy; one DMA per 4 blocks."""
                t0 = b * T
                state = {}

                def unit(nb):
                    def f():
                        if "fs" not in state:
                            state["fs"] = fsb.tile([128, NNB, TB], BF16, tag="fs", name=f"fs_{b}_{tb}")
                        fs = state["fs"]
                        nbs = slice(nb * 128, (nb + 1) * 128)
                        uid[0] += 1
                        fo = fop_ref[0].tile(
                            [128, TB], F32, tag="fo", name=f"fo_{uid[0]}"
                        )
                        nc.tensor.matmul(
                            fo, lhsT=wo_sb[:, 0, nbs], rhs=oT0,
                            start=True, stop=False,
                        )
                        nc.tensor.matmul(
                            fo, lhsT=wo_sb[:, 1, nbs], rhs=oT1,
                            start=False, stop=True,
                        )
                        if fs_on_dve[0] or nb % 2 == 0:
                            nc.vector.tensor_copy(fs[:, nb, :], fo)
                        else:
                            nc.scalar.activation(
                                fs[:, nb, :], fo, mybir.ActivationFunctionType.Copy
                            )
                        if nb % 4 == 3:
                            nc.sync.dma_start(
                                out=out_v[:, nb - 3 : nb + 1,
                                          t0 + tb * TB : t0 + (tb + 1) * TB],
                                in_=fs[:, nb - 3 : nb + 1, :],
                            )
                    return f

                return [unit(nb) for nb in range(NNB)]

            xbs_all = {}
            for b in range(B):
                t0 = b * T

                # per-(h,tb) tiles so readers only wait on the exact
                # producer (Tile dep tracking is whole-tile granular)
                qTs, kTs, vNs, oTs = {}, {}, {}, {}
                for tb in range(NTB_B):
                    for h in range(HPC):
                        qTs[h, tb] = big.tile(
                            [128, TB], BF16, tag=f"qT{h}{tb}", name=f"qT_{h}_{tb}"
                        )
                        kTs[h, tb] = big.tile(
                            [128, TB], BF16, tag=f"kT{h}{tb}", name=f"kT_{h}_{tb}"
                        )
                        oTs[h, tb] = obig.tile(
                            [128, TB], BF16, tag=f"oT{h}{tb}", name=f"oT_{h}_{tb}"
                        )
                    vNs[tb] = big.tile(
                        [128, TB // 128, M_PC], BF16, tag=f"vN{tb}", name=f"vN_{tb}"
                    )

                # ---------------- projections + RoPE ----------------
                with (
                    tc.tile_pool(name="qk_ps", bufs=2, space="PSUM") as qkp,
                    tc.tile_pool(name="v_ps", bufs=2, space="PSUM") as vp,
                ):
                    xbs = xbs_all.setdefault(b, {})
                    for tb in range(NTB_B):
                        ts_l = slice(tb * TB, (tb + 1) * TB)       # in-batch
                        tg = slice(t0 + ts_l.start, t0 + ts_l.stop)  # global
                        if tb not in xbs:
                            xbs[tb] = xpool.tile([128, NMC, TB], BF16, tag="xb", name=f"xb_{b}_{tb}")
                            if b == 0 and tb == 0:
                                # head: interleave weight + x chunks so the
                                # first matmuls start after ~3 us of DMA
                                for cs in (
                                    slice(0, 1), slice(1, 4),
                                    slice(4, 8), slice(8, 12), slice(12, 16),
                                ):
                                    nc.sync.dma_start(
                                        out=xbs[tb][:, cs, :], in_=xT_v[:, cs, tg]
                                    )
                                    nc.sync.dma_start(
                                        out=wq_sb[:, cs, :], in_=wq_v[:, cs, :]
                                    )
                                    nc.sync.dma_start(
                                        out=wk_sb[:, cs, :], in_=wk_v[:, cs, :]
                                    )
                                nc.sync.dma_start(out=wv_sb, in_=wv_v)
                                xbs[1] = xpool.tile(
                                    [128, NMC, TB], BF16, tag="xb", name="xb_0_1"
                                )
                                nc.sync.dma_start(
                                    out=xbs[1], in_=xT_v[:, :, t0 + TB : t0 + 2 * TB]
                                )
                                nc.sync.dma_start(out=cos_sb, in_=cos_d[:, :])
                                nc.sync.dma_start(out=sinh_sb, in_=sinh_d[:, :])
                            else:
                                nc.sync.dma_start(out=xbs[tb], in_=xT_v[:, :, tg])
                        xb = xbs[tb]

                        for h in range(HPC):
                            hs = slice(h * HD, (h + 1) * HD)
                            psq = qkp.tile([128, TB], F32, tag="q")
                            psk = qkp.tile([128, TB], F32, tag="k")
                            for mc in range(NMC):
                                nc.tensor.matmul(
                                    psq, lhsT=wq_sb[:, mc, hs], rhs=xb[:, mc, :],
                                    start=(mc == 0), stop=(mc == NMC - 1),
                                )
                                nc.tensor.matmul(
                                    psk, lhsT=wk_sb[:, mc, hs], rhs=xb[:, mc, :],
                                    start=(mc == 0), stop=(mc == NMC - 1),
                                )
                            emit_filler(2)
                            # RoPE: bf16 cast on Act, half-swap on DVE
                            for ps, dest in ((psq, qTs[h, tb]), (psk, kTs[h, tb])):
                                raw = rp.tile([128, TB], BF16, tag="raw")
                                nc.scalar.activation(
                                    raw, ps, mybir.ActivationFunctionType.Copy
                                )
                                shuf = rp.tile([128, TB], BF16, tag="shuf")
                                nc.vector.stream_shuffle(shuf, raw, SWAP_MASK)
                                t1 = rp.tile([128, TB], BF16, tag="t1")
                                nc.vector.tensor_mul(t1, raw, cos_sb[:, ts_l])
                                t2 = rp.tile([128, TB], BF16, tag="t2")
                                nc.vector.tensor_mul(t2, shuf, sinh_sb[:, ts_l])
                                nc.vector.tensor_add(dest, t1, t2)

                        # V directly in natural [t, (h d)] layout
                        for s in range(TB // 128):
                            j = tb * (TB // 128) + s
                            sl = slice(s * 128, (s + 1) * 128)
                            vps = vp.tile([128, M_PC], F32, tag="v")
                            for mc in range(NMC):
                                nc.tensor.matmul(
                                    vps, lhsT=xb[:, mc, sl], rhs=wv_sb[:, mc, :],
                                    start=(mc == 0), stop=(mc == NMC - 1),
                                )
                            nc.scalar.activation(
                                vNs[tb][:, s, :], vps,
                                mybir.ActivationFunctionType.Copy,
                            )
                            emit_filler(1)

                        # prefetch next x block / tail DMAs
                        if tb + 1 < NTB_B and (tb + 1) not in xbs:
                            nt = tb + 1
                            xbs[nt] = xpool.tile([128, NMC, TB], BF16, tag="xb", name=f"xb_{b}_{nt}")
                            nc.sync.dma_start(
                                out=xbs[nt],
                                in_=xT_v[:, :, t0 + nt * TB : t0 + (nt + 1) * TB],
                            )
                        if b == 0 and tb == 0:
                            nc.sync.dma_start(out=negmT, in_=negmT_d[:, :])
                            nc.sync.dma_start(out=ident, in_=id_d[:, :])
                            nc.sync.dma_start(out=wo_sb, in_=wo_v)

                # ---------------- attention (+ out-proj filler) ----------------
                with (
                    tc.tile_pool(name="st_ps", bufs=3, space="PSUM") as stp,
                    tc.tile_pool(name="po_ps", bufs=2, space="PSUM") as pop,
                    tc.tile_pool(name="z_ps", bufs=1, space="PSUM") as zpp,
                ):
                    fs_on_dve[0] = True
                    ztails = []

                    def make_ztail(po, zrb, oT_dst):
                        def f():
                            uid[0] += 1
                            zb = fop_ref[0].tile(
                                [128, TB], F32, tag="fo", name=f"zb_{uid[0]}"
                            )
                            nc.tensor.matmul(
                                zb, lhsT=ones_row, rhs=zrb, start=True, stop=True
                            )
                            zbr = zsb.tile(
                                [128, TB], BF16, tag="zbr", name=f"zbr_{uid[0]}"
                            )
                            nc.vector.tensor_copy(zbr, zb)
                            nc.vector.tensor_mul(oT_dst, po, zbr)
                        return f

                    for qg in range(NTB_B):
                        qs = slice(qg * TB, (qg + 1) * TB)
                        jmax = (TB // 128) * (qg + 1)
                        for h in range(HPC):
                            hs = slice(h * HD, (h + 1) * HD)
                            po = pop.tile([128, TB], F32, tag="po")
                            zrow = zpp.tile([1, TB], F32, tag="zrow")
                            # zrow reduction units: adjacent equal-width pt
                            # tiles are pre-added on the DVE (bf16 2x mode) so
                            # the PE ones-matmul streams half the columns
                            def zstart(j):
                                return 128 * max(0, j - 4 * qg)
                            zu = []
                            j = 0
                            while j < jmax:
                                if j + 1 < jmax and zstart(j) == zstart(j + 1):
                                    zu.append((j, j + 1))
                                    j += 2
                                else:
                                    zu.append((j, None))
                                    j += 1
                            zidx = 0
                            zready = []
                            pts = {}
                            for jj in range(jmax + LOOK + 1):
                                if ztails:
                                    ztails.pop(0)()
                                if jj < jmax:
                                    j = jj
                                    start = zstart(j)
                                    diag = j >= 4 * qg
                                    st = stp.tile([128, TB], F32, tag="st")
                                    nc.tensor.matmul(
                                        st[:, start:],
                                        lhsT=kTs[h, j // 4][
                                            :, (j % 4) * 128 : (j % 4 + 1) * 128
                                        ],
                                        rhs=qTs[h, qg][:, start:],
                                        start=True, stop=not diag,
                                        skip_group_check=True,
                                    )
                                    if diag:
                                        # causal triangle added on the PE:
                                        # M = (M^T)^T @ I, one 128-col matmul
                                        nc.tensor.matmul(
                                            st[:, start : start + 128],
                                            lhsT=negmT, rhs=ident,
                                            start=False, stop=True,
                                            skip_group_check=True,
                                        )
                                    pt = asb.tile([128, TB], BF16, tag="pt")
                                    nc.scalar.activation(
                                        pt[:, start:], st[:, start:],
                                        mybir.ActivationFunctionType.Exp,
                                        scale=SCALE,
                                    )
                                    pts[j] = (pt, start)
                                emit_filler(1)
                                # emit one pending zrow matmul (one-step lag
                                # behind the pair-add for DVE latency)
                                if zready:
                                    rhs_t, s0, first, last = zready.pop(0)
                                    nc.tensor.matmul(
                                        zrow[:, s0:], lhsT=ones_col,
                                        rhs=rhs_t[:, s0:],
                                        start=first, stop=last,
                                        skip_group_check=True,
                                    )
                                jd = jj - LOOK
                                if 0 <= jd < jmax:
                                    pt, start = pts[jd]
                                    nc.tensor.matmul(
                                        po[:, start:],
                                        lhsT=vNs[jd // 4][:, jd % 4, hs],
                                        rhs=pt[:, start:],
                                        start=(jd == 0), stop=(jd == jmax - 1),
                                        skip_group_check=True,
                                    )
                                    if zidx < len(zu):
                                        ja, jb = zu[zidx]
                                        if (jb or ja) == jd:
                                            s0 = zstart(ja)
                                            if jb is None:
                                                rhs_t = pts[ja][0]
                                            else:
                                                uid[0] += 1
                                                rhs_t = asb.tile(
                                                    [128, TB], BF16, tag="pt2",
                                                    name=f"pt2_{uid[0]}",
                                                )
                                                nc.vector.tensor_add(
                                                    rhs_t[:, s0:],
                                                    pts[ja][0][:, s0:],
                                                    pts[jb][0][:, s0:],
                                                )
                                            zready.append(
                                                (rhs_t, s0, zidx == 0,
                                                 zidx == len(zu) - 1)
                                            )
                                            zidx += 1
                                while zready and jj == jmax + LOOK:
                                    rhs_t, s0, first, last = zready.pop(0)
                                    nc.tensor.matmul(
                                        zrow[:, s0:], lhsT=ones_col,
                                        rhs=rhs_t[:, s0:],
                                        start=first, stop=last,
                                        skip_group_check=True,
                                    )
                            # normalization: recip now; the 1/Z broadcast +
                            # oT move are deferred into the next group so the
                            # PE never waits on the DVE reciprocal chain
                            zrs = zsb.tile([1, TB], F32, tag="zrs")
                            nc.vector.reciprocal(zrs, zrow)
                            zrb = zsb.tile([1, TB], BF16, tag="zrb")
                            nc.vector.tensor_copy(zrb, zrs)
                            ztails.append(make_ztail(po, zrb, oTs[h, qg]))
                        if b == B - 1 and qg == NTB_B - 1:
                            final_oT = (oTs[0, qg], oTs[1, qg])
                        else:
                            filler_q.extend(
                                outproj_units(b, qg, oTs[0, qg], oTs[1, qg])
                            )
                        if qg == 2 and b + 1 < B:
                            nxb = xpool.tile(
                                [128, NMC, TB], BF16, tag="xb", name=f"xb_{b+1}_0"
                            )
                            nc.sync.dma_start(
                                out=nxb, in_=xT_v[:, :, (b + 1) * T : (b + 1) * T + TB]
                            )
                            xbs_all.setdefault(b + 1, {})[0] = nxb
                    while ztails:
                        ztails.pop(0)()
                    fs_on_dve[0] = False

            # tail: flush leftovers, then the final block with its two
            # matmuls per row-block staggered (mmA depends only on head 0's
            # attention output, so the PE keeps running while the last
            # normalization chain completes for head 1)
            with tc.tile_pool(name="tail_ps", bufs=6, space="PSUM") as tailp:
                fop_ref[0] = tailp
                emit_filler(1 << 30)
                t0 = (B - 1) * T
                tb = NTB_B - 1
                tbs = slice(tb * TB, (tb + 1) * TB)
                fs = fsb.tile([128, NNB, TB], BF16, tag="fs", name="fs_final")
                fos = {}
                STAG = 3

                def mmA(nb):
                    uid[0] += 1
                    pool = fop if nb < 2 else tailp
                    fos[nb] = pool.tile(
                        [128, TB], F32, tag="fo", name=f"fo_fin_{uid[0]}"
                    )
                    nc.tensor.matmul(
                        fos[nb], lhsT=wo_sb[:, 0, nb * 128 : (nb + 1) * 128],
                        rhs=final_oT[0], start=True, stop=False,
                    )

                def mmB(nb):
                    nc.tensor.matmul(
                        fos[nb], lhsT=wo_sb[:, 1, nb * 128 : (nb + 1) * 128],
                        rhs=final_oT[1], start=False, stop=True,
                    )
                    if nb % 2 == 0:
                        nc.vector.tensor_copy(fs[:, nb, :], fos[nb])
                    else:
                        nc.scalar.activation(
                            fs[:, nb, :], fos[nb],
                            mybir.ActivationFunctionType.Copy,
                        )
                    if nb % 2 == 1:
                        nc.sync.dma_start(
                            out=out_v[:, nb - 1 : nb + 1,
                                      t0 + tbs.start : t0 + tbs.stop],
                            in_=fs[:, nb - 1 : nb + 1, :],
                        )

                for nb in range(NNB):
                    mmA(nb)
                    if nb >= STAG:
                        mmB(nb - STAG)
                for nb in range(NNB - STAG, NNB):
                    mmB(nb)
    _legalize_waits(nc)
    return nc


_NC_CACHE = None


def _get_program():
    global _NC_CACHE
    if _NC_CACHE is None:
        _NC_CACHE = build_program()
    return _NC_CACHE


# head_dim interleave: new row i holds old row IL_SRC[i]
IL_SRC = np.empty(HD, dtype=np.int64)
IL_SRC[0::2] = np.arange(HD // 2)
IL_SRC[1::2] = np.arange(HD // 2) + HD // 2


def _rope_tables():
    inv_freq = 1.0 / (ROPE_THETA ** (np.arange(0, HD, 2, dtype=np.float32) / HD))
    freqs = np.arange(T, dtype=np.float32)[:, None] * inv_freq[None, :]  # (T, 64)
    emb = np.concatenate([freqs, freqs], axis=-1)                        # (T, 128)
    cosT = np.cos(emb).T.astype(np.float32)                              # [128, T]
    sinT = np.sin(emb).T.astype(np.float32)
    sinhT = np.concatenate([-sinT[: HD // 2], sinT[HD // 2 :]], axis=0)
    cos_il = np.ascontiguousarray(cosT[IL_SRC]).astype(BF16_NP)
    sinh_il = np.ascontiguousarray(sinhT[IL_SRC]).astype(BF16_NP)
    return cos_il, sinh_il


def _permute_head_cols(w):
    """w: [D, M_PC] (columns = per-head head_dim blocks); apply the
    interleave permutation within each head's 128 columns."""
    out = np.empty_like(w)
    for h in range(HPC):
        blk = w[:, h * HD : (h + 1) * HD]
        out[:, h * HD : (h + 1) * HD] = blk[:, IL_SRC]
    return out


def kernel(x, Wq, Wk, Wv, Wo, **run_kwargs):
    x = np.asarray(x, dtype=np.float32)
    Wq = np.asarray(Wq, dtype=np.float32)
    Wk = np.asarray(Wk, dtype=np.float32)
    Wv = np.asarray(Wv, dtype=np.float32)
    Wo = np.asarray(Wo, dtype=np.float32)

    nc = _get_program()
    cos_il, sinh_il = _rope_tables()
    xT = np.ascontiguousarray(x.reshape(BT, D).T).astype(BF16_NP)  # [D, BT]
    # S^T[tk, tq] causal mask for diagonal blocks: keep where tq(col) >= tk(row)
    r = np.arange(128)
    negmM = np.where(r[None, :] >= r[:, None], 0.0, -1e30).astype(np.float32)
    negmTM = np.ascontiguousarray(negmM.T).astype(BF16_NP)
    identM = np.eye(128, dtype=BF16_NP)

    in_maps = []
    for c in range(NCORES):
        sl = slice(c * M_PC, (c + 1) * M_PC)
        in_maps.append(
            {
                "xT": xT,
                "negmTM": negmTM,
                "identM": identM,
                "wqT": _permute_head_cols(
                    np.ascontiguousarray(Wq[sl, :].T)
                ).astype(BF16_NP),
                "wkT": _permute_head_cols(
                    np.ascontiguousarray(Wk[sl, :].T)
                ).astype(BF16_NP),
                "wvT": np.ascontiguousarray(Wv[sl, :].T).astype(BF16_NP),
                "woT": np.ascontiguousarray(Wo[:, sl].T).astype(BF16_NP),
                "cosT": cos_il,
                "sinhT": sinh_il,
            }
        )

    res = run_bass_kernel_spmd(nc, in_maps, list(range(NCORES)), **run_kwargs)
    acc = np.zeros((D, BT), dtype=np.float32)
    for c in range(NCORES):
        acc += np.asarray(res.results[c]["partialT"], dtype=np.float32)
    out = np.ascontiguousarray(acc.T).reshape(B, T, D)
    if run_kwargs:
        return out, res
    return out


# revision 29
# speedup vs baseline: 1.4379x; 1.0346x over previous
"""Multi-head self-attention (B=2, T=2048, D=2048, H=16, RoPE, causal)
as a Bass/Tile kernel running SPMD on 8 trn2 NeuronCores.

Sharding: tensor-parallel over heads (2 heads per core). Each core
computes its heads' Q/K/V projections, RoPE, causal attention, and a
partial out-projection over its 256 feature columns; the host sums the
8 partial outputs (all-reduce equivalent).

Dataflow (per core, per batch):
  - q/k projections in "T-layout" (feature dim on partitions, time on
    free); V projected directly into natural [t, d] layout by using the
    x tile as the stationary operand (no PE transposes).
  - RoPE with the head_dim rows interleaved as (r, r+64) pairs, a
    permutation folded into Wq/Wk and the cos/sin tables host-side (the
    QK^T contraction is invariant); the half-rotation is then a swap of
    adjacent partitions done with a DVE stream-shuffle (no PE matmul).
  - scores computed directly transposed: S^T[tk, tq] = K_j^T.T @ Q,
    one 128-contraction matmul per (key-chunk, 512-wide q-group), so the
    exp output P^T is immediately the PV matmul's moving operand.
    Column-blocks that the causal mask fully zeroes are skipped (matmul/
    exp widths narrowed to the valid range). Softmax skips
    max-subtraction (scores are O(10)).
  - row sums Z via a [128,1] ones matmul accumulated in PSUM across key
    chunks; reciprocal on DVE; partition-broadcast via a bf16 ones
    matmul; normalization fused with the PSUM->SBUF move of the PV
    result.
  - out-projection (bf16 partial result) is emitted as filler work
    inside the next q-group's attention loops to keep the PE busy while
    exp results are in flight; host sums partials across cores.
"""

import sys

sys.path.insert(0, "/opt/trn_rl_repo")

import ml_dtypes
import numpy as np

import concourse.bass as bass
import concourse.mybir as mybir
import concourse.tile as tile
from concourse.bass_utils import run_bass_kernel_spmd
from concourse.tile_rust import add_dep_helper


def _legalize_waits(nc):
    """Walrus codegen rejects >2 sync waits on DMA/matmul/nop-class
    instructions, and Tile's pool-recycle waits bypass its own elision.
    Spill excess waits (>1) onto freshly inserted same-engine NoOps
    placed immediately before the offending instruction (sound w.r.t.
    per-engine program order)."""
    spill_id = [0]
    for bb in nc.m.functions[0].blocks:
        new_insts = []
        for inst in bb.instructions:
            si = getattr(inst, "sync_info", None)
            if si is None or not si.on_wait:
                new_insts.append(inst)
                continue
            eng = getattr(inst, "engine", None)
            kept = list(si.on_wait)
            if len(kept) > 1 and eng is not None:
                excess, kept = kept[:-1], kept[-1:]
                for w in excess:
                    spill_id[0] += 1
                    nop = mybir.InstNoOp(
                        name=f"I-wspill-{spill_id[0]}",
                        ins=[],
                        outs=[],
                        engine=eng,
                    )
                    nop.sync_info = mybir.SyncInfo(on_wait=[w], on_update=[])
                    new_insts.append(nop)
                si.on_wait[:] = kept
            new_insts.append(inst)
        if len(new_insts) != len(bb.instructions):
            bb.instructions[:] = new_insts


B, T, D, H, HD = 2, 2048, 2048, 16, 128
NCORES = 8
HPC = H // NCORES            # heads per core = 2
M_PC = HPC * HD              # per-core feature slice = 256
BT = B * T                   # 4096
SCALE = HD ** -0.5
ROPE_THETA = 10000.0

F32 = mybir.dt.float32
BF16 = mybir.dt.bfloat16
BF16_NP = ml_dtypes.bfloat16

TB = 512                     # t-block for projections / q-groups
NTB_B = T // TB              # 4 t-blocks per batch
NMC = D // 128               # 16 contraction chunks
NKC = T // 128               # 16 key chunks per batch
NNB = D // 128               # 16 out-proj row blocks
LOOK = 2                     # attention software-pipeline lookahead

# swap adjacent partitions within each 32-partition quadrant
SWAP_MASK = [i + 1 if i % 2 == 0 else i - 1 for i in range(32)]


def build_program():
    nc = bass.Bass()

    xT_d = nc.declare_dram_parameter("xT", [D, BT], BF16, isOutput=False)
    negmT_d = nc.declare_dram_parameter("negmTM", [128, 128], BF16, isOutput=False)
    id_d = nc.declare_dram_parameter("identM", [128, 128], BF16, isOutput=False)
    wq_d = nc.declare_dram_parameter("wqT", [D, M_PC], BF16, isOutput=False)
    wk_d = nc.declare_dram_parameter("wkT", [D, M_PC], BF16, isOutput=False)
    wv_d = nc.declare_dram_parameter("wvT", [D, M_PC], BF16, isOutput=False)
    wo_d = nc.declare_dram_parameter("woT", [M_PC, D], BF16, isOutput=False)
    cos_d = nc.declare_dram_parameter("cosT", [HD, T], BF16, isOutput=False)
    sinh_d = nc.declare_dram_parameter("sinhT", [HD, T], BF16, isOutput=False)
    out_d = nc.declare_dram_parameter("partialT", [D, BT], BF16, isOutput=True)

    xT_v = xT_d.rearrange("(c p) t -> p c t", p=128)      # [128, 16, BT]
    wq_v = wq_d.rearrange("(c p) n -> p c n", p=128)      # [128, 16, 256]
    wk_v = wk_d.rearrange("(c p) n -> p c n", p=128)
    wv_v = wv_d.rearrange("(c p) n -> p c n", p=128)
    wo_v = wo_d.rearrange("(c p) n -> p c n", p=128)      # [128, 2, 2048]
    out_v = out_d.rearrange("(c p) t -> p c t", p=128)    # [128, 16, BT]

    with tile.TileContext(nc) as tc:
        with (
            tc.tile_pool(name="wpool", bufs=1) as wpool,
            tc.tile_pool(name="big", bufs=1) as big,
            tc.tile_pool(name="obig", bufs=2) as obig,
            tc.tile_pool(name="xpool", bufs=2) as xpool,
            tc.tile_pool(name="rp", bufs=2) as rp,
            tc.tile_pool(name="attn_sb", bufs=4) as asb,
            tc.tile_pool(name="z_sb", bufs=2) as zsb,
            tc.tile_pool(name="fs_sb", bufs=2) as fsb,
            tc.tile_pool(name="fo_ps", bufs=2, space="PSUM") as fop,
        ):
            # ---- persistent weights / tables ----
            wq_sb = wpool.tile([128, NMC, M_PC], BF16, tag="wq")
            wk_sb = wpool.tile([128, NMC, M_PC], BF16, tag="wk")
            wv_sb = wpool.tile([128, NMC, M_PC], BF16, tag="wv")
            wo_sb = wpool.tile([128, HPC, D], BF16, tag="wo")
            cos_sb = wpool.tile([128, T], BF16, tag="cos")
            sinh_sb = wpool.tile([128, T], BF16, tag="sinh")
            negmT = wpool.tile([128, 128], BF16, tag="negmT")
            ident = wpool.tile([128, 128], BF16, tag="ident")
            ones_col = wpool.tile([128, 1], BF16, tag="ones_c")
            nc.vector.memset(ones_col, 1.0)
            ones_row = wpool.tile([1, 128], BF16, tag="ones_r")
            nc.vector.memset(ones_row, 1.0)
            # pre-warm the Act engine's Exp table during the head DMAs
            warm = wpool.tile([1, 2], F32, tag="warm")
            nc.vector.memset(warm, 0.0)
            warm2 = wpool.tile([1, 2], BF16, tag="warm2")
            nc.scalar.activation(warm2, warm, mybir.ActivationFunctionType.Exp)

            # ---- out-projection filler machinery ----
            filler_q = []
            vq = []
            fop_ref = [fop]
            fs_on_dve = [False]
            uid = [0]

            def emit_filler(n, reserve=0):
                for _ in range(n):
                    if vq:
                        vq.pop(0)()
                    elif len(filler_q) > reserve:
                        filler_q.pop(0)()
                    else:
                        break

            def outproj_units(b, tb, oT0, oT1):
                """16 PE-units computing the partial out-projection for
                time block (b, tb); each unit = 2 accumulating matmuls +
                a PSUM->SBUF bf16 copy; one DMA per 4 blocks."""
                t0 = b * T
                state = {}

                def unit(nb):
                    def f():
                        if "fs" not in state:
                            state["fs"] = fsb.tile([128, NNB, TB], BF16, tag="fs", name=f"fs_{b}_{tb}")
                        fs = state["fs"]
                        nbs = slice(nb * 128, (nb + 1) * 128)
                        uid[0] += 1
                        fo = fop_ref[0].tile(
                            [128, TB], F32, tag="fo", name=f"fo_{uid[0]}"
                        )
                        nc.tensor.matmul(
                            fo, lhsT=wo_sb[:, 0, nbs], rhs=oT0,
                            start=True, stop=False,
                        )
                        nc.tensor.matmul(
                            fo, lhsT=wo_sb[:, 1, nbs], rhs=oT1,
                            start=False, stop=True,
                        )
                        if fs_on_dve[0] or nb % 2 == 0:
                            nc.vector.tensor_copy(fs[:, nb, :], fo)
                        else:
                            nc.scalar.activation(
                                fs[:, nb, :], fo, mybir.ActivationFunctionType.Copy
                            )
                        if nb % 4 == 3:
                            nc.sync.dma_start(
                                out=out_v[:, nb - 3 : nb + 1,
                                          t0 + tb * TB : t0 + (tb + 1) * TB],
                                in_=fs[:, nb - 3 : nb + 1, :],
                            )
                    return f

                return [unit(nb) for nb in range(NNB)]

            xbs_all = {}
            preproj_raws = []
            for b in range(B):
                t0 = b * T

                # per-(h,tb) tiles so readers only wait on the exact
                # producer (Tile dep tracking is whole-tile granular)
                qTs, kTs, vNs, oTs = {}, {}, {}, {}
                for tb in range(NTB_B):
                    for h in range(HPC):
                        qTs[h, tb] = big.tile(
                            [128, TB], BF16, tag=f"qT{h}{tb}", name=f"qT_{h}_{tb}"
                        )
                        kTs[h, tb] = big.tile(
                            [128, TB], BF16, tag=f"kT{h}{tb}", name=f"kT_{h}_{tb}"
                        )
                        oTs[h, tb] = obig.tile(
                            [128, TB], BF16, tag=f"oT{h}{tb}", name=f"oT_{h}_{tb}"
                        )
                    vNs[tb] = big.tile(
                        [128, TB // 128, M_PC], BF16, tag=f"vN{tb}", name=f"vN_{tb}"
                    )

                # ---------------- projections + RoPE ----------------
                with (
                    tc.tile_pool(name="qk_ps", bufs=2, space="PSUM") as qkp,
                    tc.tile_pool(name="v_ps", bufs=2, space="PSUM") as vp,
                ):
                    xbs = xbs_all.setdefault(b, {})
                    for tb in range(NTB_B):
                        ts_l = slice(tb * TB, (tb + 1) * TB)       # in-batch
                        tg = slice(t0 + ts_l.start, t0 + ts_l.stop)  # global
                        if tb not in xbs:
                            xbs[tb] = xpool.tile([128, NMC, TB], BF16, tag="xb", name=f"xb_{b}_{tb}")
                            if b == 0 and tb == 0:
                                # head: interleave weight + x chunks so the
                                # first matmuls start after ~3 us of DMA
                                for cs in (
                                    slice(0, 1), slice(1, 4),
                                    slice(4, 8), slice(8, 12), slice(12, 16),
                                ):
                                    nc.sync.dma_start(
                                        out=xbs[tb][:, cs, :], in_=xT_v[:, cs, tg]
                                    )
                                    nc.sync.dma_start(
                                        out=wq_sb[:, cs, :], in_=wq_v[:, cs, :]
                                    )
                                    nc.sync.dma_start(
                                        out=wk_sb[:, cs, :], in_=wk_v[:, cs, :]
                                    )
                                nc.sync.dma_start(out=wv_sb, in_=wv_v)
                                xbs[1] = xpool.tile(
                                    [128, NMC, TB], BF16, tag="xb", name="xb_0_1"
                                )
                                nc.sync.dma_start(
                                    out=xbs[1], in_=xT_v[:, :, t0 + TB : t0 + 2 * TB]
                                )
                                nc.sync.dma_start(out=cos_sb, in_=cos_d[:, :])
                                nc.sync.dma_start(out=sinh_sb, in_=sinh_d[:, :])
                            else:
                                nc.sync.dma_start(out=xbs[tb], in_=xT_v[:, :, tg])
                        xb = xbs[tb]

                        for h in range(HPC):
                            hs = slice(h * HD, (h + 1) * HD)
                            pre_raws = None
                            if tb == 0 and h == 0 and preproj_raws:
                                pre_raws = preproj_raws.pop(0)
                            else:
                                psq = qkp.tile([128, TB], F32, tag="q")
                                psk = qkp.tile([128, TB], F32, tag="k")
                                for mc in range(NMC):
                                    nc.tensor.matmul(
                                        psq, lhsT=wq_sb[:, mc, hs],
                                        rhs=xb[:, mc, :],
                                        start=(mc == 0), stop=(mc == NMC - 1),
                                    )
                                    nc.tensor.matmul(
                                        psk, lhsT=wk_sb[:, mc, hs],
                                        rhs=xb[:, mc, :],
                                        start=(mc == 0), stop=(mc == NMC - 1),
                                    )
                            emit_filler(2)
                            # RoPE: bf16 cast on Act, half-swap on DVE
                            for qk_i, dest in ((0, qTs[h, tb]), (1, kTs[h, tb])):
                                if pre_raws is not None:
                                    raw = pre_raws[qk_i]
                                else:
                                    ps = psq if qk_i == 0 else psk
                                    raw = rp.tile([128, TB], BF16, tag="raw")
                                    nc.scalar.activation(
                                        raw, ps, mybir.ActivationFunctionType.Copy
                                    )
                                shuf = rp.tile([128, TB], BF16, tag="shuf")
                                nc.vector.stream_shuffle(shuf, raw, SWAP_MASK)
                                t1 = rp.tile([128, TB], BF16, tag="t1")
                                nc.vector.tensor_mul(t1, raw, cos_sb[:, ts_l])
                                t2 = rp.tile([128, TB], BF16, tag="t2")
                                nc.vector.tensor_mul(t2, shuf, sinh_sb[:, ts_l])
                                nc.vector.tensor_add(dest, t1, t2)

                        # V directly in natural [t, (h d)] layout; the last
                        # time block's V is deferred into the attention phase
                        # as filler work (it is only read from q-group 3)
                        if tb < NTB_B - 1:
                            for s in range(TB // 128):
                                sl = slice(s * 128, (s + 1) * 128)
                                vps = vp.tile([128, M_PC], F32, tag="v")
                                for mc in range(NMC):
                                    nc.tensor.matmul(
                                        vps, lhsT=xb[:, mc, sl],
                                        rhs=wv_sb[:, mc, :],
                                        start=(mc == 0), stop=(mc == NMC - 1),
                                    )
                                nc.scalar.activation(
                                    vNs[tb][:, s, :], vps,
                                    mybir.ActivationFunctionType.Copy,
                                )
                                emit_filler(1)
                        else:
                            def v_unit(s, xb=xb, vN=vNs[tb]):
                                def f():
                                    uid[0] += 1
                                    vps = fop_ref[0].tile(
                                        [128, TB], F32, tag="fo",
                                        name=f"vfo_{uid[0]}",
                                    )
                                    for mc in range(NMC):
                                        nc.tensor.matmul(
                                            vps[:, :M_PC], lhsT=xb[:, mc,
                                            s * 128 : (s + 1) * 128],
                                            rhs=wv_sb[:, mc, :],
                                            start=(mc == 0), stop=(mc == NMC - 1),
                                        )
                                    nc.scalar.activation(
                                        vN[:, s, :], vps[:, :M_PC],
                                        mybir.ActivationFunctionType.Copy,
                                    )
                                return f

                            for s in range(TB // 128):
                                vq.append(v_unit(s))

                        # prefetch next x block / tail DMAs
                        if tb + 1 < NTB_B and (tb + 1) not in xbs:
                            nt = tb + 1
                            xbs[nt] = xpool.tile([128, NMC, TB], BF16, tag="xb", name=f"xb_{b}_{nt}")
                            nc.sync.dma_start(
                                out=xbs[nt],
                                in_=xT_v[:, :, t0 + nt * TB : t0 + (nt + 1) * TB],
                            )
                        if b == 0 and tb == 0:
                            nc.sync.dma_start(out=negmT, in_=negmT_d[:, :])
                            nc.sync.dma_start(out=ident, in_=id_d[:, :])
                            nc.sync.dma_start(out=wo_sb, in_=wo_v)

                # ---------------- attention (+ out-proj filler) ----------------
                with (
                    tc.tile_pool(name="st_ps", bufs=3, space="PSUM") as stp,
                    tc.tile_pool(name="po_ps", bufs=2, space="PSUM") as pop,
                    tc.tile_pool(name="z_ps", bufs=1, space="PSUM") as zpp,
                ):
                    ztails = []

                    def make_ztail(po, zrb, oT_dst):
                        def f():
                            uid[0] += 1
                            zb = fop_ref[0].tile(
                                [128, TB], F32, tag="fo", name=f"zb_{uid[0]}"
                            )
                            nc.tensor.matmul(
                                zb, lhsT=ones_row, rhs=zrb, start=True, stop=True
                            )
                            zbr = zsb.tile(
                                [128, TB], BF16, tag="zbr", name=f"zbr_{uid[0]}"
                            )
                            nc.vector.tensor_copy(zbr, zb)
                            nc.vector.tensor_mul(oT_dst, po, zbr)
                        return f

                    for qg in range(NTB_B):
                        if qg == NTB_B - 1:
                            while vq:
                                vq.pop(0)()
                        qs = slice(qg * TB, (qg + 1) * TB)
                        jmax = (TB // 128) * (qg + 1)
                        for h in range(HPC):
                            hs = slice(h * HD, (h + 1) * HD)
                            po = pop.tile([128, TB], F32, tag="po")
                            zrow = zpp.tile([1, TB], F32, tag="zrow")
                            # zrow reduction units: adjacent equal-width pt
                            # tiles are pre-added on the DVE (bf16 2x mode) so
                            # the PE ones-matmul streams half the columns
                            def zstart(j):
                                return 128 * max(0, j - 4 * qg)
                            zu = []   # each unit: tuple of 1, 2 or 4 j's
                            j = 0
                            while j < jmax:
                                w = 1
                                while (
                                    w < 4 and j + w < jmax
                                    and zstart(j) == zstart(j + w)
                                ):
                                    w += 1
                                w = {1: 1, 2: 2, 3: 2, 4: 4}[w]
                                zu.append(tuple(range(j, j + w)))
                                j += w
                            zidx = 0
                            zready = []
                            zpair = [None]
                            pts = {}
                            for jj in range(jmax + LOOK + 1):
                                if ztails:
                                    ztails.pop(0)()
                                if jj < jmax:
                                    j = jj
                                    start = zstart(j)
                                    diag = j >= 4 * qg
                                    st = stp.tile([128, TB], F32, tag="st")
                                    nc.tensor.matmul(
                                        st[:, start:],
                                        lhsT=kTs[h, j // 4][
                                            :, (j % 4) * 128 : (j % 4 + 1) * 128
                                        ],
                                        rhs=qTs[h, qg][:, start:],
                                        start=True, stop=not diag,
                                        skip_group_check=True,
                                    )
                                    if diag:
                                        # causal triangle added on the PE:
                                        # M = (M^T)^T @ I, one 128-col matmul
                                        nc.tensor.matmul(
                                            st[:, start : start + 128],
                                            lhsT=negmT, rhs=ident,
                                            start=False, stop=True,
                                            skip_group_check=True,
                                        )
                                    pt = asb.tile([128, TB], BF16, tag="pt")
                                    nc.scalar.activation(
                                        pt[:, start:], st[:, start:],
                                        mybir.ActivationFunctionType.Exp,
                                        scale=SCALE,
                                    )
                                    pts[j] = (pt, start)
                                emit_filler(
                                    1,
                                    reserve=2 if (
                                        b == B - 1 and qg == NTB_B - 1
                                    ) else 0,
                                )
                                # emit one pending zrow matmul (one-step lag
                                # behind the pair-add for DVE latency)
                                if zready and zready[0][4] <= jj:
                                    rhs_t, s0, first, last, _ = zready.pop(0)
                                    nc.tensor.matmul(
                                        zrow[:, s0:], lhsT=ones_col,
                                        rhs=rhs_t[:, s0:],
                                        start=first, stop=last,
                                        skip_group_check=True,
                                    )
                                jd = jj - LOOK
                                if 0 <= jd < jmax:
                                    pt, start = pts[jd]
                                    nc.tensor.matmul(
                                        po[:, start:],
                                        lhsT=vNs[jd // 4][:, jd % 4, hs],
                                        rhs=pt[:, start:],
                                        start=(jd == 0), stop=(jd == jmax - 1),
                                        skip_group_check=True,
                                    )
                                    if zidx < len(zu):
                                        js = zu[zidx]
                                        s0 = zstart(js[0])
                                        if len(js) >= 2 and jd == js[1]:
                                            uid[0] += 1
                                            zpair[0] = asb.tile(
                                                [128, TB], BF16, tag="pt2",
                                                name=f"pt2_{uid[0]}",
                                            )
                                            nc.vector.tensor_add(
                                                zpair[0][:, s0:],
                                                pts[js[0]][0][:, s0:],
                                                pts[js[1]][0][:, s0:],
                                            )
                                        if jd == js[-1]:
                                            if len(js) == 1:
                                                rhs_t = pts[js[0]][0]
                                            elif len(js) == 2:
                                                rhs_t = zpair[0]
                                            else:
                                                uid[0] += 1
                                                cd = asb.tile(
                                                    [128, TB], BF16, tag="pt2",
                                                    name=f"pt2_{uid[0]}",
                                                )
                                                nc.vector.tensor_add(
                                                    cd[:, s0:],
                                                    pts[js[2]][0][:, s0:],
                                                    pts[js[3]][0][:, s0:],
                                                )
                                                uid[0] += 1
                                                rhs_t = asb.tile(
                                                    [128, TB], BF16, tag="pt4",
                                                    name=f"pt4_{uid[0]}",
                                                )
                                                nc.vector.tensor_add(
                                                    rhs_t[:, s0:],
                                                    zpair[0][:, s0:], cd[:, s0:],
                                                )
                                            zready.append(
                                                (rhs_t, s0, zidx == 0,
                                                 zidx == len(zu) - 1,
                                                 jj + 1)
                                            )
                                            zidx += 1
                                while zready and jj == jmax + LOOK:
                                    rhs_t, s0, first, last, _ = zready.pop(0)
                                    nc.tensor.matmul(
                                        zrow[:, s0:], lhsT=ones_col,
                                        rhs=rhs_t[:, s0:],
                                        start=first, stop=last,
                                        skip_group_check=True,
                                    )
                            # normalization: recip now; the 1/Z broadcast +
                            # oT move are deferred into the next group so the
                            # PE never waits on the DVE reciprocal chain
                            zrs = zsb.tile([1, TB], F32, tag="zrs")
                            nc.vector.reciprocal(zrs, zrow)
                            zrb = zsb.tile([1, TB], BF16, tag="zrb")
                            nc.vector.tensor_copy(zrb, zrs)
                            ztails.append(make_ztail(po, zrb, oTs[h, qg]))
                        if b == B - 1 and qg == NTB_B - 1:
                            final_oT = (oTs[0, qg], oTs[1, qg])
                        else:
                            filler_q.extend(
                                outproj_units(b, qg, oTs[0, qg], oTs[1, qg])
                            )
                        if qg == 2 and b + 1 < B:
                            nxb = xpool.tile(
                                [128, NMC, TB], BF16, tag="xb", name=f"xb_{b+1}_0"
                            )
                            nc.sync.dma_start(
                                out=nxb, in_=xT_v[:, :, (b + 1) * T : (b + 1) * T + TB]
                            )
                            xbs_all.setdefault(b + 1, {})[0] = nxb
                    # cover the last normalization chain's latency with
                    # ready PE work (the next batch's first q/k projection
                    # group) before flushing the final z-tails; the bf16 raw
                    # copies are emitted immediately so the fo-pool slot
                    # recycling sees the readers
                    if b + 1 < B:
                        nxb = xbs_all[b + 1][0]
                        raws = {}
                        for nm, wsb in (("q", wq_sb), ("k", wk_sb)):
                            uid[0] += 1
                            pre = fop_ref[0].tile(
                                [128, TB], F32, tag="fo", name=f"pre{nm}_{uid[0]}"
                            )
                            for mc in range(NMC):
                                nc.tensor.matmul(
                                    pre, lhsT=wsb[:, mc, 0:HD],
                                    rhs=nxb[:, mc, :],
                                    start=(mc == 0), stop=(mc == NMC - 1),
                                )
                            if nm == "q":
                                while ztails:
                                    ztails.pop(0)()
                            raws[nm] = rp.tile(
                                [128, TB], BF16, tag=f"pr{nm}",
                                name=f"praw_{nm}_{uid[0]}",
                            )
                            nc.scalar.activation(
                                raws[nm], pre, mybir.ActivationFunctionType.Copy
                            )
                        preproj_raws.append((raws["q"], raws["k"]))
                    else:
                        emit_filler(1)
                        while ztails:
                            ztails.pop(0)()
                            emit_filler(1)
                    fs_on_dve[0] = False

            # tail: flush leftovers, then the final block with its two
            # matmuls per row-block staggered (mmA depends only on head 0's
            # attention output, so the PE keeps running while the last
            # normalization chain completes for head 1)
            with tc.tile_pool(name="tail_ps", bufs=6, space="PSUM") as tailp:
                fop_ref[0] = tailp
                emit_filler(1 << 30)
                t0 = (B - 1) * T
                tb = NTB_B - 1
                tbs = slice(tb * TB, (tb + 1) * TB)
                fs = fsb.tile([128, NNB, TB], BF16, tag="fs", name="fs_final")
                fos = {}
                STAG = 3

                def mmA(nb):
                    uid[0] += 1
                    pool = fop if nb < 2 else tailp
                    fos[nb] = pool.tile(
                        [128, TB], F32, tag="fo", name=f"fo_fin_{uid[0]}"
                    )
                    nc.tensor.matmul(
                        fos[nb], lhsT=wo_sb[:, 0, nb * 128 : (nb + 1) * 128],
                        rhs=final_oT[0], start=True, stop=False,
                    )

                def mmB(nb):
                    nc.tensor.matmul(
                        fos[nb], lhsT=wo_sb[:, 1, nb * 128 : (nb + 1) * 128],
                        rhs=final_oT[1], start=False, stop=True,
                    )
                    if nb % 2 == 0:
                        nc.vector.tensor_copy(fs[:, nb, :], fos[nb])
                    else:
                        nc.scalar.activation(
                            fs[:, nb, :], fos[nb],
                            mybir.ActivationFunctionType.Copy,
                        )
                    if nb % 2 == 1:
                        nc.sync.dma_start(
                            out=out_v[:, nb - 1 : nb + 1,
                                      t0 + tbs.start : t0 + tbs.stop],
                            in_=fs[:, nb - 1 : nb + 1, :],
                        )

                for nb in range(NNB):
                    mmA(nb)
                    if nb >= STAG:
                        mmB(nb - STAG)
                for nb in range(NNB - STAG, NNB):
                    mmB(nb)
    _legalize_waits(nc)
    return nc


_NC_CACHE = None


def _get_program():
    global _NC_CACHE
    if _NC_CACHE is None:
        _NC_CACHE = build_program()
    return _NC_CACHE


# head_dim interleave: new row i holds old row IL_SRC[i]
IL_SRC = np.empty(HD, dtype=np.int64)
IL_SRC[0::2] = np.arange(HD // 2)
IL_SRC[1::2] = np.arange(HD // 2) + HD // 2


def _rope_tables():
    inv_freq = 1.0 / (ROPE_THETA ** (np.arange(0, HD, 2, dtype=np.float32) / HD))
    freqs = np.arange(T, dtype=np.float32)[:, None] * inv_freq[None, :]  # (T, 64)
    emb = np.concatenate([freqs, freqs], axis=-1)                        # (T, 128)
    cosT = np.cos(emb).T.astype(np.float32)                              # [128, T]
    sinT = np.sin(emb).T.astype(np.float32)
    sinhT = np.concatenate([-sinT[: HD // 2], sinT[HD // 2 :]], axis=0)
    cos_il = np.ascontiguousarray(cosT[IL_SRC]).astype(BF16_NP)
    sinh_il = np.ascontiguousarray(sinhT[IL_SRC]).astype(BF16_NP)
    return cos_il, sinh_il


def _permute_head_cols(w):
    """w: [D, M_PC] (columns = per-head head_dim blocks); apply the
    interleave permutation within each head's 128 columns."""
    out = np.empty_like(w)
    for h in range(HPC):
        blk = w[:, h * HD : (h + 1) * HD]
        out[:, h * HD : (h + 1) * HD] = blk[:, IL_SRC]
    return out


def kernel(x, Wq, Wk, Wv, Wo, **run_kwargs):
    x = np.asarray(x, dtype=np.float32)
    Wq = np.asarray(Wq, dtype=np.float32)
    Wk = np.asarray(Wk, dtype=np.float32)
    Wv = np.asarray(Wv, dtype=np.float32)
    Wo = np.asarray(Wo, dtype=np.float32)

    nc = _get_program()
    cos_il, sinh_il = _rope_tables()
    xT = np.ascontiguousarray(x.reshape(BT, D).T).astype(BF16_NP)  # [D, BT]
    # S^T[tk, tq] causal mask for diagonal blocks: keep where tq(col) >= tk(row)
    r = np.arange(128)
    negmM = np.where(r[None, :] >= r[:, None], 0.0, -1e30).astype(np.float32)
    negmTM = np.ascontiguousarray(negmM.T).astype(BF16_NP)
    identM = np.eye(128, dtype=BF16_NP)

    in_maps = []
    for c in range(NCORES):
        sl = slice(c * M_PC, (c + 1) * M_PC)
        in_maps.append(
            {
                "xT": xT,
                "negmTM": negmTM,
                "identM": identM,
                "wqT": _permute_head_cols(
                    np.ascontiguousarray(Wq[sl, :].T)
                ).astype(BF16_NP),
                "wkT": _permute_head_cols(
                    np.ascontiguousarray(Wk[sl, :].T)
                ).astype(BF16_NP),
                "wvT": np.ascontiguousarray(Wv[sl, :].T).astype(BF16_NP),
                "woT": np.ascontiguousarray(Wo[:, sl].T).astype(BF16_NP),
                "cosT": cos_il,
                "sinhT": sinh_il,
            }
        )

    res = run_bass_kernel_spmd(nc, in_maps, list(range(NCORES)), **run_kwargs)
    acc = np.zeros((D, BT), dtype=np.float32)
    for c in range(NCORES):
        acc += np.asarray(res.results[c]["partialT"], dtype=np.float32)
    out = np.ascontiguousarray(acc.T).reshape(B, T, D)
    if run_kwargs:
        return out, res
    return out


# revision 47
# speedup vs baseline: 1.4849x; 1.0327x over previous
"""Multi-head self-attention (B=2, T=2048, D=2048, H=16, RoPE, causal)
as a Bass/Tile kernel running SPMD on 8 trn2 NeuronCores.

Sharding: tensor-parallel over heads (2 heads per core). Each core
computes its heads' Q/K/V projections, RoPE, causal attention, and a
partial out-projection over its 256 feature columns; the host sums the
8 partial outputs (all-reduce equivalent).

Dataflow (per core, per batch):
  - q/k projections in "T-layout" (feature dim on partitions, time on
    free); V projected directly into natural [t, d] layout by using the
    x tile as the stationary operand (no PE transposes).
  - RoPE with the head_dim rows interleaved as (r, r+64) pairs, a
    permutation folded into Wq/Wk and the cos/sin tables host-side (the
    QK^T contraction is invariant); the half-rotation is then a swap of
    adjacent partitions done with a DVE stream-shuffle (no PE matmul).
  - scores computed directly transposed: S^T[tk, tq] = K_j^T.T @ Q,
    one 128-contraction matmul per (key-chunk, 512-wide q-group), so the
    exp output P^T is immediately the PV matmul's moving operand.
    Column-blocks that the causal mask fully zeroes are skipped (matmul/
    exp widths narrowed to the valid range). Softmax skips
    max-subtraction (scores are O(10)).
  - row sums Z: exp tiles are accumulated in-place on the DVE (bf16 2x
    mode) into one tile per q-group, reduced by a single [128,1] ones
    matmul; reciprocal on DVE; partition-broadcast via a bf16 ones
    matmul; the broadcast + normalization are deferred into the next
    group so the PE never waits on the reciprocal chain.
  - out-projection (bf16 partial result) is emitted as filler work
    inside the next q-group's attention loops to keep the PE busy while
    exp results are in flight; the last time block's V projection is
    deferred the same way; host sums partials across cores.
"""

import sys

sys.path.insert(0, "/opt/trn_rl_repo")

import ml_dtypes
import numpy as np

import concourse.bass as bass
import concourse.mybir as mybir
import concourse.tile as tile
from concourse.bass_utils import run_bass_kernel_spmd


def _legalize_waits(nc):
    """Walrus codegen rejects >2 sync waits on DMA/matmul/nop-class
    instructions, and Tile's pool-recycle waits bypass its own elision.
    Spill excess waits (>1) onto freshly inserted same-engine NoOps
    placed immediately before the offending instruction (sound w.r.t.
    per-engine program order)."""
    spill_id = [0]
    for bb in nc.m.functions[0].blocks:
        new_insts = []
        for inst in bb.instructions:
            si = getattr(inst, "sync_info", None)
            if si is None or not si.on_wait:
                new_insts.append(inst)
                continue
            eng = getattr(inst, "engine", None)
            kept = list(si.on_wait)
            if len(kept) > 1 and eng is not None:
                excess, kept = kept[:-1], kept[-1:]
                for w in excess:
                    spill_id[0] += 1
                    nop = mybir.InstNoOp(
                        name=f"I-wspill-{spill_id[0]}",
                        ins=[],
                        outs=[],
                        engine=eng,
                    )
                    nop.sync_info = mybir.SyncInfo(on_wait=[w], on_update=[])
                    new_insts.append(nop)
                si.on_wait[:] = kept
            new_insts.append(inst)
        if len(new_insts) != len(bb.instructions):
            bb.instructions[:] = new_insts


B, T, D, H, HD = 2, 2048, 2048, 16, 128
NCORES = 8
HPC = H // NCORES            # heads per core = 2
M_PC = HPC * HD              # per-core feature slice = 256
BT = B * T                   # 4096
SCALE = HD ** -0.5
ROPE_THETA = 10000.0

F32 = mybir.dt.float32
BF16 = mybir.dt.bfloat16
BF16_NP = ml_dtypes.bfloat16

TB = 512                     # t-block for projections / q-groups
NTB_B = T // TB              # 4 t-blocks per batch
NMC = D // 128               # 16 contraction chunks
NKC = T // 128               # 16 key chunks per batch
NNB = D // 128               # 16 out-proj row blocks
LOOK = 2                     # attention software-pipeline lookahead

# swap adjacent partitions within each 32-partition quadrant
SWAP_MASK = [i + 1 if i % 2 == 0 else i - 1 for i in range(32)]


def build_program():
    nc = bass.Bass()

    xT_d = nc.declare_dram_parameter("xT", [D, BT], BF16, isOutput=False)
    negmT_d = nc.declare_dram_parameter("negmTM", [128, 128], BF16, isOutput=False)
    id_d = nc.declare_dram_parameter("identM", [128, 128], BF16, isOutput=False)
    wq_d = nc.declare_dram_parameter("wqT", [D, M_PC], BF16, isOutput=False)
    wk_d = nc.declare_dram_parameter("wkT", [D, M_PC], BF16, isOutput=False)
    wv_d = nc.declare_dram_parameter("wvT", [D, M_PC], BF16, isOutput=False)
    wo_d = nc.declare_dram_parameter("woT", [M_PC, D], BF16, isOutput=False)
    cos_d = nc.declare_dram_parameter("cosT", [HD, T], BF16, isOutput=False)
    sinh_d = nc.declare_dram_parameter("sinhT", [HD, T], BF16, isOutput=False)
    out_d = nc.declare_dram_parameter("partialT", [D, BT], BF16, isOutput=True)

    xT_v = xT_d.rearrange("(c p) t -> p c t", p=128)      # [128, 16, BT]
    wq_v = wq_d.rearrange("(c p) n -> p c n", p=128)      # [128, 16, 256]
    wk_v = wk_d.rearrange("(c p) n -> p c n", p=128)
    wv_v = wv_d.rearrange("(c p) n -> p c n", p=128)
    wo_v = wo_d.rearrange("(c p) n -> p c n", p=128)      # [128, 2, 2048]
    out_v = out_d.rearrange("(c p) t -> p c t", p=128)    # [128, 16, BT]

    with tile.TileContext(nc) as tc:
        with (
            tc.tile_pool(name="wpool", bufs=1) as wpool,
            tc.tile_pool(name="big", bufs=1) as big,
            tc.tile_pool(name="obig", bufs=2) as obig,
            tc.tile_pool(name="xpool", bufs=2) as xpool,
            tc.tile_pool(name="rp", bufs=2) as rp,
            tc.tile_pool(name="attn_sb", bufs=4) as asb,
            tc.tile_pool(name="z_sb", bufs=2) as zsb,
            tc.tile_pool(name="fs_sb", bufs=2) as fsb,
            tc.tile_pool(name="fo_ps", bufs=2, space="PSUM") as fop,
        ):
            # ---- persistent weights / tables ----
            wq_sb = wpool.tile([128, NMC, M_PC], BF16, tag="wq")
            wk_sb = wpool.tile([128, NMC, M_PC], BF16, tag="wk")
            wv_sb = wpool.tile([128, NMC, M_PC], BF16, tag="wv")
            wo_sb = wpool.tile([128, HPC, D], BF16, tag="wo")
            cos_sb = wpool.tile([128, T], BF16, tag="cos")
            sinh_sb = wpool.tile([128, T], BF16, tag="sinh")
            negmT = wpool.tile([128, 128], BF16, tag="negmT")
            ident = wpool.tile([128, 128], BF16, tag="ident")
            ones_col = wpool.tile([128, 1], BF16, tag="ones_c")
            nc.vector.memset(ones_col, 1.0)
            ones_row = wpool.tile([1, 128], BF16, tag="ones_r")
            nc.vector.memset(ones_row, 1.0)
            # pre-warm the Act engine's Exp table during the head DMAs
            warm = wpool.tile([1, 2], F32, tag="warm")
            nc.vector.memset(warm, 0.0)
            warm2 = wpool.tile([1, 2], BF16, tag="warm2")
            nc.scalar.activation(warm2, warm, mybir.ActivationFunctionType.Exp)

            # ---- out-projection filler machinery ----
            filler_q = []
            vq = []
            fop_ref = [fop]
            fs_on_dve = [False]
            uid = [0]

            def emit_filler(n, reserve=0):
                for _ in range(n):
                    if vq:
                        vq.pop(0)()
                    elif len(filler_q) > reserve:
                        filler_q.pop(0)()
                    else:
                        break

            def outproj_units(b, tb, oT0, oT1):
                """16 PE-units computing the partial out-projection for
                time block (b, tb); each unit = 2 accumulating matmuls +
                a PSUM->SBUF bf16 copy; one DMA per 4 blocks."""
                t0 = b * T
                state = {}

                def unit(nb):
                    def f():
                        if "fs" not in state:
                            state["fs"] = fsb.tile([128, NNB, TB], BF16, tag="fs", name=f"fs_{b}_{tb}")
                        fs = state["fs"]
                        nbs = slice(nb * 128, (nb + 1) * 128)
                        uid[0] += 1
                        fo = fop_ref[0].tile(
                            [128, TB], F32, tag="fo", name=f"fo_{uid[0]}"
                        )
                        nc.tensor.matmul(
                            fo, lhsT=wo_sb[:, 0, nbs], rhs=oT0,
                            start=True, stop=False,
                        )
                        nc.tensor.matmul(
                            fo, lhsT=wo_sb[:, 1, nbs], rhs=oT1,
                            start=False, stop=True,
                        )
                        if fs_on_dve[0] or nb % 2 == 0:
                            nc.vector.tensor_copy(fs[:, nb, :], fo)
                        else:
                            nc.scalar.activation(
                                fs[:, nb, :], fo, mybir.ActivationFunctionType.Copy
                            )
                        if nb % 4 == 3:
                            nc.sync.dma_start(
                                out=out_v[:, nb - 3 : nb + 1,
                                          t0 + tb * TB : t0 + (tb + 1) * TB],
                                in_=fs[:, nb - 3 : nb + 1, :],
                            )
                    return f

                return [unit(nb) for nb in range(NNB)]

            xbs_all = {}
            preproj_raws = []
            for b in range(B):
                t0 = b * T

                # per-(h,tb) tiles so readers only wait on the exact
                # producer (Tile dep tracking is whole-tile granular)
                qTs, kTs, vNs, oTs = {}, {}, {}, {}
                for tb in range(NTB_B):
                    for h in range(HPC):
                        qTs[h, tb] = big.tile(
                            [128, TB], BF16, tag=f"qT{h}{tb}", name=f"qT_{h}_{tb}"
                        )
                        kTs[h, tb] = big.tile(
                            [128, TB], BF16, tag=f"kT{h}{tb}", name=f"kT_{h}_{tb}"
                        )
                        oTs[h, tb] = obig.tile(
                            [128, TB], BF16, tag=f"oT{h}{tb}", name=f"oT_{h}_{tb}"
                        )
                    vNs[tb] = big.tile(
                        [128, TB // 128, M_PC], BF16, tag=f"vN{tb}", name=f"vN_{tb}"
                    )

                # ---------------- projections + RoPE ----------------
                with (
                    tc.tile_pool(name="qk_ps", bufs=2, space="PSUM") as qkp,
                    tc.tile_pool(name="v_ps", bufs=2, space="PSUM") as vp,
                ):
                    xbs = xbs_all.setdefault(b, {})
                    for tb in range(NTB_B):
                        ts_l = slice(tb * TB, (tb + 1) * TB)       # in-batch
                        tg = slice(t0 + ts_l.start, t0 + ts_l.stop)  # global
                        if tb not in xbs:
                            xbs[tb] = xpool.tile([128, NMC, TB], BF16, tag="xb", name=f"xb_{b}_{tb}")
                            if b == 0 and tb == 0:
                                # head: interleave weight + x chunks so the
                                # first matmuls start after ~3 us of DMA
                                for cs in (
                                    slice(0, 1), slice(1, 4),
                                    slice(4, 8), slice(8, 12), slice(12, 16),
                                ):
                                    nc.sync.dma_start(
                                        out=xbs[tb][:, cs, :], in_=xT_v[:, cs, tg]
                                    )
                                    nc.sync.dma_start(
                                        out=wq_sb[:, cs, :], in_=wq_v[:, cs, :]
                                    )
                                    nc.sync.dma_start(
                                        out=wk_sb[:, cs, :], in_=wk_v[:, cs, :]
                                    )
                                nc.sync.dma_start(out=wv_sb, in_=wv_v)
                                xbs[1] = xpool.tile(
                                    [128, NMC, TB], BF16, tag="xb", name="xb_0_1"
                                )
                                nc.sync.dma_start(
                                    out=xbs[1], in_=xT_v[:, :, t0 + TB : t0 + 2 * TB]
                                )
                                nc.sync.dma_start(out=cos_sb, in_=cos_d[:, :])
                                nc.sync.dma_start(out=sinh_sb, in_=sinh_d[:, :])
                            else:
                                nc.sync.dma_start(out=xbs[tb], in_=xT_v[:, :, tg])
                        xb = xbs[tb]

                        for h in range(HPC):
                            hs = slice(h * HD, (h + 1) * HD)
                            pre_raws = None
                            if tb == 0 and h == 0 and preproj_raws:
                                pre_raws = preproj_raws.pop(0)
                            else:
                                psq = qkp.tile([128, TB], F32, tag="q")
                                psk = qkp.tile([128, TB], F32, tag="k")
                                for mc in range(NMC):
                                    nc.tensor.matmul(
                                        psq, lhsT=wq_sb[:, mc, hs],
                                        rhs=xb[:, mc, :],
                                        start=(mc == 0), stop=(mc == NMC - 1),
                                    )
                                    nc.tensor.matmul(
                                        psk, lhsT=wk_sb[:, mc, hs],
                                        rhs=xb[:, mc, :],
                                        start=(mc == 0), stop=(mc == NMC - 1),
                                    )
                            emit_filler(2)
                            # RoPE: bf16 cast on Act, half-swap on DVE
                            for qk_i, dest in ((0, qTs[h, tb]), (1, kTs[h, tb])):
                                if pre_raws is not None:
                                    raw = pre_raws[qk_i]
                                else:
                                    ps = psq if qk_i == 0 else psk
                                    raw = rp.tile([128, TB], BF16, tag="raw")
                                    nc.scalar.activation(
                                        raw, ps, mybir.ActivationFunctionType.Copy
                                    )
                                shuf = rp.tile([128, TB], BF16, tag="shuf")
                                nc.vector.stream_shuffle(shuf, raw, SWAP_MASK)
                                t1 = rp.tile([128, TB], BF16, tag="t1")
                                nc.vector.tensor_mul(t1, raw, cos_sb[:, ts_l])
                                t2 = rp.tile([128, TB], BF16, tag="t2")
                                nc.vector.tensor_mul(t2, shuf, sinh_sb[:, ts_l])
                                nc.vector.tensor_add(dest, t1, t2)

                        # V directly in natural [t, (h d)] layout; the last
                        # time block's V is deferred into the attention phase
                        # as filler work (it is only read from q-group 3)
                        if tb < NTB_B - 1:
                            for s in range(TB // 128):
                                sl = slice(s * 128, (s + 1) * 128)
                                vps = vp.tile([128, M_PC], F32, tag="v")
                                for mc in range(NMC):
                                    nc.tensor.matmul(
                                        vps, lhsT=xb[:, mc, sl],
                                        rhs=wv_sb[:, mc, :],
                                        start=(mc == 0), stop=(mc == NMC - 1),
                                    )
                                nc.scalar.activation(
                                    vNs[tb][:, s, :], vps,
                                    mybir.ActivationFunctionType.Copy,
                                )
                                emit_filler(1)
                        else:
                            def v_unit(s, xb=xb, vN=vNs[tb]):
                                def f():
                                    uid[0] += 1
                                    vps = fop_ref[0].tile(
                                        [128, TB], F32, tag="fo",
                                        name=f"vfo_{uid[0]}",
                                    )
                                    for mc in range(NMC):
                                        nc.tensor.matmul(
                                            vps[:, :M_PC], lhsT=xb[:, mc,
                                            s * 128 : (s + 1) * 128],
                                            rhs=wv_sb[:, mc, :],
                                            start=(mc == 0), stop=(mc == NMC - 1),
                                        )
                                    nc.vector.tensor_copy(
                                        vN[:, s, :], vps[:, :M_PC]
                                    )
                                return f

                            for s in range(TB // 128):
                                vq.append(v_unit(s))

                        # prefetch next x block / tail DMAs
                        if tb + 1 < NTB_B and (tb + 1) not in xbs:
                            nt = tb + 1
                            xbs[nt] = xpool.tile([128, NMC, TB], BF16, tag="xb", name=f"xb_{b}_{nt}")
                            nc.sync.dma_start(
                                out=xbs[nt],
                                in_=xT_v[:, :, t0 + nt * TB : t0 + (nt + 1) * TB],
                            )
                        if b == 0 and tb == 0:
                            nc.sync.dma_start(out=negmT, in_=negmT_d[:, :])
                            nc.sync.dma_start(out=ident, in_=id_d[:, :])
                            nc.sync.dma_start(out=wo_sb, in_=wo_v)

                # ---------------- attention (+ out-proj filler) ----------------
                with (
                    tc.tile_pool(name="st_ps", bufs=3, space="PSUM") as stp,
                    tc.tile_pool(name="po_ps", bufs=2, space="PSUM") as pop,
                    tc.tile_pool(name="z_ps", bufs=1, space="PSUM") as zpp,
                ):
                    ztails = []

                    def make_ztail(po, zrb, oT_dst):
                        def f():
                            uid[0] += 1
                            zb = fop_ref[0].tile(
                                [128, TB], F32, tag="fo", name=f"zb_{uid[0]}"
                            )
                            nc.tensor.matmul(
                                zb, lhsT=ones_row, rhs=zrb, start=True, stop=True
                            )
                            zbr = zsb.tile(
                                [128, TB], BF16, tag="zbr", name=f"zbr_{uid[0]}"
                            )
                            nc.vector.tensor_copy(zbr, zb)
                            nc.vector.tensor_mul(oT_dst, po, zbr)
                        return f

                    for qg in range(NTB_B):
                        if qg == NTB_B - 1:
                            while vq:
                                vq.pop(0)()
                        qs = slice(qg * TB, (qg + 1) * TB)
                        jmax = (TB // 128) * (qg + 1)
                        for h in range(HPC):
                            hs = slice(h * HD, (h + 1) * HD)
                            po = pop.tile([128, TB], F32, tag="po")
                            zrow = zpp.tile([1, TB], F32, tag="zrow")
                            # Z row sums: all exp tiles are accumulated
                            # in-place on the DVE (bf16 2x mode) into one
                            # tile, so the PE streams a single ones-matmul
                            # per group
                            def zstart(j):
                                return 128 * max(0, j - 4 * qg)
                            racc = asb.tile(
                                [128, TB], BF16, tag="pt2", name=f"racc_{b}_{qg}_{h}"
                            )
                            pts = {}
                            for jj in range(jmax + LOOK + 1):
                                if jj < jmax:
                                    j = jj
                                    start = zstart(j)
                                    diag = j >= 4 * qg
                                    st = stp.tile([128, TB], F32, tag="st")
                                    nc.tensor.matmul(
                                        st[:, start:],
                                        lhsT=kTs[h, j // 4][
                                            :, (j % 4) * 128 : (j % 4 + 1) * 128
                                        ],
                                        rhs=qTs[h, qg][:, start:],
                                        start=True, stop=not diag,
                                        skip_group_check=True,
                                    )
                                    if diag:
                                        # causal triangle added on the PE:
                                        # M = (M^T)^T @ I, one 128-col matmul
                                        nc.tensor.matmul(
                                            st[:, start : start + 128],
                                            lhsT=negmT, rhs=ident,
                                            start=False, stop=True,
                                            skip_group_check=True,
                                        )
                                    pt = asb.tile([128, TB], BF16, tag="pt")
                                    nc.scalar.activation(
                                        pt[:, start:], st[:, start:],
                                        mybir.ActivationFunctionType.Exp,
                                        scale=SCALE,
                                    )
                                    pts[j] = (pt, start)
                                if ztails:
                                    ztails.pop(0)()
                                emit_filler(
                                    1,
                                    reserve=4 if (
                                        b == B - 1 and qg == NTB_B - 1
                                    ) else 0,
                                )
                                jd = jj - LOOK
                                if 0 <= jd < jmax:
                                    pt, start = pts[jd]
                                    nc.tensor.matmul(
                                        po[:, start:],
                                        lhsT=vNs[jd // 4][:, jd % 4, hs],
                                        rhs=pt[:, start:],
                                        start=(jd == 0), stop=(jd == jmax - 1),
                                        skip_group_check=True,
                                    )
                                    s0 = zstart(jd)
                                    if jd == 0:
                                        nc.vector.tensor_copy(
                                            racc, pts[0][0]
                                        )
                                    else:
                                        nc.vector.tensor_add(
                                            racc[:, s0:], racc[:, s0:],
                                            pts[jd][0][:, s0:],
                                        )
                                if jj == jmax + LOOK:
                                    nc.tensor.matmul(
                                        zrow, lhsT=ones_col, rhs=racc,
                                        start=True, stop=True,
                                    )
                            # normalization: recip now; the 1/Z broadcast +
                            # oT move are deferred into the next group so the
                            # PE never waits on the DVE reciprocal chain
                            zrs = zsb.tile([1, TB], F32, tag="zrs")
                            nc.vector.reciprocal(zrs, zrow)
                            zrb = zsb.tile([1, TB], BF16, tag="zrb")
                            nc.vector.tensor_copy(zrb, zrs)
                            ztails.append(make_ztail(po, zrb, oTs[h, qg]))
                        if b == B - 1 and qg == NTB_B - 1:
                            final_oT = (oTs[0, qg], oTs[1, qg])
                        else:
                            filler_q.extend(
                                outproj_units(b, qg, oTs[0, qg], oTs[1, qg])
                            )
                        if qg == 2 and b + 1 < B:
                            nxb = xpool.tile(
                                [128, NMC, TB], BF16, tag="xb", name=f"xb_{b+1}_0"
                            )
                            nc.sync.dma_start(
                                out=nxb, in_=xT_v[:, :, (b + 1) * T : (b + 1) * T + TB]
                            )
                            xbs_all.setdefault(b + 1, {})[0] = nxb
                    # cover the last normalization chain's latency with
                    # ready PE work (the next batch's first q/k projection
                    # group) before flushing the final z-tails; the bf16 raw
                    # copies are emitted immediately so the fo-pool slot
                    # recycling sees the readers
                    if b + 1 < B:
                        nxb = xbs_all[b + 1][0]
                        raws = {}
                        for nm, wsb in (("q", wq_sb), ("k", wk_sb)):
                            uid[0] += 1
                            pre = fop_ref[0].tile(
                                [128, TB], F32, tag="fo", name=f"pre{nm}_{uid[0]}"
                            )
                            for mc in range(NMC):
                                nc.tensor.matmul(
                                    pre, lhsT=wsb[:, mc, 0:HD],
                                    rhs=nxb[:, mc, :],
                                    start=(mc == 0), stop=(mc == NMC - 1),
                                )
                            if nm == "q":
                                while ztails:
                                    ztails.pop(0)()
                            raws[nm] = rp.tile(
                                [128, TB], BF16, tag=f"pr{nm}",
                                name=f"praw_{nm}_{uid[0]}",
                            )
                            nc.scalar.activation(
                                raws[nm], pre, mybir.ActivationFunctionType.Copy
                            )
                        preproj_raws.append((raws["q"], raws["k"]))
                    else:
                        emit_filler(3)
                        while ztails:
                            ztails.pop(0)()
                            emit_filler(1)
                    fs_on_dve[0] = False

            # tail: flush leftovers, then the final block with its two
            # matmuls per row-block staggered (mmA depends only on head 0's
            # attention output, so the PE keeps running while the last
            # normalization chain completes for head 1)
            with tc.tile_pool(name="tail_ps", bufs=6, space="PSUM") as tailp:
                fop_ref[0] = tailp
                emit_filler(1 << 30)
                t0 = (B - 1) * T
                tb = NTB_B - 1
                tbs = slice(tb * TB, (tb + 1) * TB)
                fs = fsb.tile([128, NNB, TB], BF16, tag="fs", name="fs_final")
                fos = {}
                STAG = 3

                def mmA(nb):
                    uid[0] += 1
                    pool = fop if nb < 2 else tailp
                    fos[nb] = pool.tile(
                        [128, TB], F32, tag="fo", name=f"fo_fin_{uid[0]}"
                    )
                    nc.tensor.matmul(
                        fos[nb], lhsT=wo_sb[:, 0, nb * 128 : (nb + 1) * 128],
                        rhs=final_oT[0], start=True, stop=False,
                    )

                def mmB(nb):
                    nc.tensor.matmul(
                        fos[nb], lhsT=wo_sb[:, 1, nb * 128 : (nb + 1) * 128],
                        rhs=final_oT[1], start=False, stop=True,
                    )
                    if nb % 2 == 0:
                        nc.vector.tensor_copy(fs[:, nb, :], fos[nb])
                    else:
                        nc.scalar.activation(
                            fs[:, nb, :], fos[nb],
                            mybir.ActivationFunctionType.Copy,
                        )
                    if nb % 2 == 1:
                        nc.sync.dma_start(
                            out=out_v[:, nb - 1 : nb + 1,
                                      t0 + tbs.start : t0 + tbs.stop],
                            in_=fs[:, nb - 1 : nb + 1, :],
                        )

                for nb in range(NNB):
                    mmA(nb)
                    if nb >= STAG:
                        mmB(nb - STAG)
                for nb in range(NNB - STAG, NNB):
                    mmB(nb)
    _legalize_waits(nc)
    return nc


_NC_CACHE = None


def _get_program():
    global _NC_CACHE
    if _NC_CACHE is None:
        _NC_CACHE = build_program()
    return _NC_CACHE


# head_dim interleave: new row i holds old row IL_SRC[i]
IL_SRC = np.empty(HD, dtype=np.int64)
IL_SRC[0::2] = np.arange(HD // 2)
IL_SRC[1::2] = np.arange(HD // 2) + HD // 2


def _rope_tables():
    inv_freq = 1.0 / (ROPE_THETA ** (np.arange(0, HD, 2, dtype=np.float32) / HD))
    freqs = np.arange(T, dtype=np.float32)[:, None] * inv_freq[None, :]  # (T, 64)
    emb = np.concatenate([freqs, freqs], axis=-1)                        # (T, 128)
    cosT = np.cos(emb).T.astype(np.float32)                              # [128, T]
    sinT = np.sin(emb).T.astype(np.float32)
    sinhT = np.concatenate([-sinT[: HD // 2], sinT[HD // 2 :]], axis=0)
    cos_il = np.ascontiguousarray(cosT[IL_SRC]).astype(BF16_NP)
    sinh_il = np.ascontiguousarray(sinhT[IL_SRC]).astype(BF16_NP)
    return cos_il, sinh_il


def _permute_head_cols(w):
    """w: [D, M_PC] (columns = per-head head_dim blocks); apply the
    interleave permutation within each head's 128 columns."""
    out = np.empty_like(w)
    for h in range(HPC):
        blk = w[:, h * HD : (h + 1) * HD]
        out[:, h * HD : (h + 1) * HD] = blk[:, IL_SRC]
    return out


def kernel(x, Wq, Wk, Wv, Wo, **run_kwargs):
    x = np.asarray(x, dtype=np.float32)
    Wq = np.asarray(Wq, dtype=np.float32)
    Wk = np.asarray(Wk, dtype=np.float32)
    Wv = np.asarray(Wv, dtype=np.float32)
    Wo = np.asarray(Wo, dtype=np.float32)

    nc = _get_program()
    cos_il, sinh_il = _rope_tables()
    xT = np.ascontiguousarray(x.reshape(BT, D).T).astype(BF16_NP)  # [D, BT]
    # S^T[tk, tq] causal mask for diagonal blocks: keep where tq(col) >= tk(row)
    r = np.arange(128)
    negmM = np.where(r[None, :] >= r[:, None], 0.0, -1e30).astype(np.float32)
    negmTM = np.ascontiguousarray(negmM.T).astype(BF16_NP)
    identM = np.eye(128, dtype=BF16_NP)

    in_maps = []
    for c in range(NCORES):
        sl = slice(c * M_PC, (c + 1) * M_PC)
        in_maps.append(
            {
                "xT": xT,
                "negmTM": negmTM,
                "identM": identM,
                "wqT": _permute_head_cols(
                    np.ascontiguousarray(Wq[sl, :].T)
                ).astype(BF16_NP),
                "wkT": _permute_head_cols(
                    np.ascontiguousarray(Wk[sl, :].T)
                ).astype(BF16_NP),
                "wvT": np.ascontiguousarray(Wv[sl, :].T).astype(BF16_NP),
                "woT": np.ascontiguousarray(Wo[:, sl].T).astype(BF16_NP),
                "cosT": cos_il,
                "sinhT": sinh_il,
            }
        )

    res = run_bass_kernel_spmd(nc, in_maps, list(range(NCORES)), **run_kwargs)
    acc = np.zeros((D, BT), dtype=np.float32)
    for c in range(NCORES):
        acc += np.asarray(res.results[c]["partialT"], dtype=np.float32)
    out = np.ascontiguousarray(acc.T).reshape(B, T, D)
    if run_kwargs:
        return out, res
    return out
